# revision 1
# baseline (speedup 1.0000x reference)
"""Trainium2 Bass kernel for batched masked-Kabsch RMSD (Coords2RMSD).

Contract: kernel(**inputs) takes FULL inputs (input [128, 49152] f32,
target [128, 49152] f32, num_atoms [128] i32) and returns the FULL
output [128] f32.  Internally shards batch rows across 8 NeuronCores
(16 rows per core), runs one SPMD Bass program, and gathers.

Device algorithm (per core):
  - Row r of this core's shard is spread over partitions 8r..8r+7
    (2048 atoms per partition, contiguous 24 KiB DMA runs).
  - Bulk phase computes 17 masked reduction channels per row:
    M[k,l] = sum_m x_k y_l (9), sx (3), sy (3), Sxx, Syy.
    Masking uses the DVE TENSOR_PAGED_MASK custom op (prefix mask from
    per-partition valid counts).  Channels are spread across DVE
    (tensor_tensor_reduce), ACT (activation w/ accum), and GPSIMD
    (scalar_tensor_tensor w/ accum).  Per-partition partials land in an
    accumulator tile; one PE matmul with a row-selector reduces
    partitions -> [16 rows, channels] in PSUM.
  - Epilogue (per row, 16 partitions): centered covariance C, E0,
    eigenvalues of C^T C via the trigonometric closed form
    (acos via arctan, cos via sin), singular values, reflection
    correction via det(C)/(s0*s1), rmsd = sqrt(max(E0-2*sum_s,0)/n+1e-8).
"""

import os
import sys

import numpy as np

for _p in ("/opt/trn_rl_repo", "/root/.axon_site/_ro/trn_rl_repo"):
    if os.path.isdir(_p) and _p not in sys.path:
        sys.path.insert(0, _p)

B = 128
MAX_ATOMS = 16384
N3 = 3 * MAX_ATOMS          # 49152
NCORES = 8
ROWS = B // NCORES          # 16 rows per core
BLOCKS = 8                  # partition blocks per row (128 partitions / 16 rows)
CHUNK = MAX_ATOMS // BLOCKS  # 2048 atoms per partition
NT = 4                      # processing tiles along the free dim
APT = CHUNK // NT           # 1024 atoms per partition per tile
EPT = 3 * APT               # 3072 elements per partition per tile
NCH = 17                    # reduction channels
ACC = NCH * NT              # accumulator columns

# aux tensor columns: [0:16] row-selector, [16:16+NT] per-tile valid counts,
# [16+NT] n (rows 0:16), [17+NT : 19+NT] cos offsets (rows 0:16)
AUXW = 16 + NT + 1 + 2
COL_VT = 16
COL_N = 16 + NT
COL_CST = 17 + NT

_state = {}


def _build():
    import concourse.bacc as bacc
    import concourse.bass as bass
    import concourse.mybir as mybir
    import concourse.tile as tile
    from concourse.dve_ops import TENSOR_PAGED_MASK

    dt = mybir.dt
    AFT = mybir.ActivationFunctionType
    ALU = mybir.AluOpType
    AX = mybir.AxisListType

    nc = bacc.Bacc("TRN2", target_bir_lowering=False, debug=False)

    x_d = nc.dram_tensor("x", [ROWS, N3], dt.float32, kind="ExternalInput").ap()
    y_d = nc.dram_tensor("y", [ROWS, N3], dt.float32, kind="ExternalInput").ap()
    m_d = nc.dram_tensor("msk", [ROWS, N3], dt.bfloat16, kind="ExternalInput").ap()
    aux_d = nc.dram_tensor("aux", [128, AUXW], dt.float32, kind="ExternalInput").ap()
    o_d = nc.dram_tensor("o", [ROWS, 1], dt.float32, kind="ExternalOutput").ap()

    # DRAM views: [ROWS, N3] -> [128, 6144]; partition p = 8*r + i holds
    # elements [6144*i, 6144*(i+1)) of row r.
    x_r = x_d.rearrange("r (i e) -> (r i) e", i=BLOCKS)
    y_r = y_d.rearrange("r (i e) -> (r i) e", i=BLOCKS)
    m_r = m_d.rearrange("r (i e) -> (r i) e", i=BLOCKS)

    with tile.TileContext(nc) as tc:
        with (
            tc.tile_pool(name="data", bufs=2) as data_pool,
            tc.tile_pool(name="maskd", bufs=2) as mask_pool,
            tc.tile_pool(name="ascr", bufs=2) as ascr_pool,
            tc.tile_pool(name="dscr", bufs=3) as dscr_pool,
            tc.tile_pool(name="pscr", bufs=3) as pscr_pool,
            tc.tile_pool(name="small", bufs=1) as small_pool,
            tc.tile_pool(name="ep", bufs=1) as ep_pool,
            tc.tile_pool(name="psum", bufs=1, space="PSUM") as psum_pool,
        ):
            aux = small_pool.tile([128, AUXW], dt.float32)
            nc.sync.dma_start(out=aux[:], in_=aux_d)
            accum = small_pool.tile([128, ACC], dt.float32)

            sel = aux[:, 0:16]
            nn = aux[0:16, COL_N : COL_N + 1]
            cst = aux[0:16, COL_CST : COL_CST + 2]

            def A(ch, t):
                return accum[:, ch * NT + t : ch * NT + t + 1]

            for t in range(NT):
                xt = data_pool.tile([128, EPT], dt.float32, tag="xt")
                yt = data_pool.tile([128, EPT], dt.float32, tag="yt")
                mt = data_pool.tile([128, EPT], dt.bfloat16, tag="mt")
                sl = slice(EPT * t, EPT * (t + 1))
                nc.sync.dma_start(out=xt[:], in_=x_r[:, sl])
                nc.sync.dma_start(out=yt[:], in_=y_r[:, sl])
                nc.sync.dma_start(out=mt[:], in_=m_r[:, sl])

                xm = mask_pool.tile([128, EPT], dt.float32, tag="xm")
                ym = mask_pool.tile([128, EPT], dt.float32, tag="ym")

                x3 = xt[:].rearrange("p (a c) -> p a c", c=3)
                y3 = yt[:].rearrange("p (a c) -> p a c", c=3)
                xm3 = xm[:].rearrange("p (a c) -> p a c", c=3)
                ym3 = ym[:].rearrange("p (a c) -> p a c", c=3)

                # masked data via host-shipped bf16 0/1 mask
                nc.vector.tensor_tensor(xm[:], xt[:], mt[:], ALU.mult)
                nc.vector.tensor_tensor(ym[:], yt[:], mt[:], ALU.mult)

                def comp(tile3, k):
                    # [128, APT] strided view of component k
                    return tile3[:, :, k : k + 1].rearrange("p a one -> p (a one)")

                # cross channels M[k,l] = ch 3k+l: 6 fused on DVE; 3 as
                # GPSIMD products reduced on ACT
                dve_ch = [(0, 0), (0, 1), (0, 2), (1, 0), (1, 1), (1, 2)]
                pool_ch = [(2, 0), (2, 1), (2, 2)]
                for (k, l) in dve_ch:
                    scr = dscr_pool.tile([128, APT], dt.float32, tag="dscr")
                    nc.vector.scalar_tensor_tensor(
                        out=scr[:], in0=comp(xm3, k), scalar=1.0,
                        in1=comp(y3, l), op0=ALU.mult, op1=ALU.mult,
                        accum_out=A(3 * k + l, t),
                    )
                import os
                use_pool = os.environ.get("K_USE_POOL", "0") == "1"
                for (k, l) in pool_ch:
                    if use_pool:
                        scr = pscr_pool.tile([128, APT], dt.float32, tag="pscr")
                        nc.gpsimd.tensor_tensor(
                            scr[:], comp(xm3, k), comp(y3, l), ALU.mult
                        )
                        scr2 = ascr_pool.tile([128, APT], dt.float32, tag="lscr")
                        nc.scalar.activation(
                            scr2[:], scr[:], AFT.Identity, accum_out=A(3 * k + l, t)
                        )
                    else:
                        scr = dscr_pool.tile([128, APT], dt.float32, tag="dscr")
                        nc.vector.scalar_tensor_tensor(
                            out=scr[:], in0=comp(xm3, k), scalar=1.0,
                            in1=comp(y3, l), op0=ALU.mult, op1=ALU.mult,
                            accum_out=A(3 * k + l, t),
                        )

                # linear sums sx_k (ch 9..11), sy_k (ch 12..14) on ACT
                for k in range(3):
                    scr = ascr_pool.tile([128, APT], dt.float32, tag="lscr")
                    nc.scalar.activation(
                        scr[:], comp(xm3, k), AFT.Identity, accum_out=A(9 + k, t)
                    )
                    scr = ascr_pool.tile([128, APT], dt.float32, tag="lscr")
                    nc.scalar.activation(
                        scr[:], comp(ym3, k), AFT.Identity, accum_out=A(12 + k, t)
                    )
                # Sxx (ch 15), Syy (ch 16) on ACT: sum Square(masked)
                scr = ascr_pool.tile([128, EPT], dt.float32, tag="qscr")
                nc.scalar.activation(scr[:], xm[:], AFT.Square, accum_out=A(15, t))
                scr = ascr_pool.tile([128, EPT], dt.float32, tag="qscr")
                nc.scalar.activation(scr[:], ym[:], AFT.Square, accum_out=A(16, t))

            # partition combine: [16, ACC] = sel.T @ accum
            stats_ps = psum_pool.tile([16, ACC], dt.float32)
            nc.tensor.matmul(stats_ps[:], sel, accum[:], start=True, stop=True)

            # ---------------- epilogue (per-row, 16 partitions) ----------
            _ep_ctr = [0]

            def ept(w):
                _ep_ctr[0] += 1
                nm = f"ep{_ep_ctr[0]}"
                return ep_pool.tile([16, w], dt.float32, name=nm, tag=nm)

            TT = nc.vector.tensor_tensor
            STT = nc.vector.scalar_tensor_tensor
            TS = nc.vector.tensor_scalar

            S = ept(NCH)
            nc.vector.tensor_reduce(
                S[:],
                stats_ps[:].rearrange("p (c t) -> p c t", t=NT),
                AX.X,
                ALU.add,
            )
            M9 = S[:, 0:9]
            sx = S[:, 9:12]
            sy = S[:, 12:15]

            rn = ept(1)
            nc.vector.reciprocal(rn[:], nn)
            nrn = ept(1)
            nc.vector.tensor_scalar_mul(nrn[:], rn[:], -1.0)

            # C = M - (sx sy^T) / n
            O9 = ept(9)
            o3 = O9[:].rearrange("p (k l) -> p k l", l=3)
            TT(o3, sx.unsqueeze(2).broadcast_to([16, 3, 3]),
               sy.unsqueeze(1).broadcast_to([16, 3, 3]), ALU.mult)
            C9 = ept(9)
            STT(C9[:], O9[:], nrn[:, 0:1], M9, ALU.mult, ALU.add)

            # E0 = Sxx + Syy - (|sx|^2 + |sy|^2)/n
            sq6 = ept(6)
            ss = ept(1)
            nc.vector.scalar_tensor_tensor(
                out=sq6[:], in0=S[:, 9:15], scalar=1.0, in1=S[:, 9:15],
                op0=ALU.mult, op1=ALU.mult, accum_out=ss[:],
            )
            sxy = ept(1)
            TT(sxy[:], S[:, 15:16], S[:, 16:17], ALU.add)
            E0 = ept(1)
            STT(E0[:], ss[:], nrn[:, 0:1], sxy[:], ALU.mult, ALU.add)

            # A = C^T C  (A[i,j] = sum_a C[3a+i] C[3a+j])
            W27 = ept(27)
            w3 = W27[:].rearrange("p (i j a) -> p i j a", j=3, a=3)
            cu = C9[:].rearrange("p (a i) -> p i a", i=3).unsqueeze(2)
            cv = C9[:].rearrange("p (a j) -> p j a", j=3).unsqueeze(1)
            TT(w3, cu.broadcast_to([16, 3, 3, 3]), cv.broadcast_to([16, 3, 3, 3]),
               ALU.mult)
            A9 = ept(9)
            nc.vector.tensor_reduce(
                A9[:].rearrange("p (i j) -> p i j", j=3), w3, AX.X, ALU.add
            )

            trA = ept(1)
            nc.vector.tensor_reduce(trA[:], A9[:, 0:9:4], AX.X, ALU.add)
            # normalize: An = A / s2, s2 = trA/3  (=> trace(An) = 3, q' = 1)
            s2 = ept(1)
            TS(s2[:], trA[:], 1.0 / 3.0, 1e-20, ALU.mult, ALU.max)
            is2 = ept(1)
            nc.vector.reciprocal(is2[:], s2[:])
            An = ept(9)
            nc.vector.tensor_scalar_mul(An[:], A9[:], is2[:, 0:1])
            f2 = ept(9)
            trA2 = ept(1)
            nc.vector.scalar_tensor_tensor(
                out=f2[:], in0=An[:], scalar=1.0, in1=An[:],
                op0=ALU.mult, op1=ALU.mult, accum_out=trA2[:],
            )
            # P2 = (trA2 - 3)/6 ; clamped
            P2 = ept(1)
            TS(P2[:], trA2[:], 1.0 / 6.0, -0.5, ALU.mult, ALU.add)
            P2c = ept(1)
            nc.vector.tensor_scalar_max(P2c[:], P2[:], 1e-30)

            # det(C) (signed, raw scale)
            PA = ept(3)
            PB = ept(3)
            TT(PA[:, 0:1], C9[:, 4:5], C9[:, 8:9], ALU.mult)
            TT(PA[:, 1:2], C9[:, 5:6], C9[:, 6:7], ALU.mult)
            TT(PA[:, 2:3], C9[:, 3:4], C9[:, 7:8], ALU.mult)
            TT(PB[:, 0:1], C9[:, 5:6], C9[:, 7:8], ALU.mult)
            TT(PB[:, 1:2], C9[:, 3:4], C9[:, 8:9], ALU.mult)
            TT(PB[:, 2:3], C9[:, 4:5], C9[:, 6:7], ALU.mult)
            cof = ept(3)
            TT(cof[:], PA[:], PB[:], ALU.subtract)
            det3 = ept(3)
            detC = ept(1)
            nc.vector.scalar_tensor_tensor(
                out=det3[:], in0=C9[:, 0:3], scalar=1.0, in1=cof[:],
                op0=ALU.mult, op1=ALU.mult, accum_out=detC[:],
            )
            # detAn = det(C)^2 / s2^3 ; D = det(An - I) = detAn + trA2/2 - 2.5
            detC2 = ept(1)
            TT(detC2[:], detC[:], detC[:], ALU.mult)
            i2 = ept(1)
            TT(i2[:], is2[:], is2[:], ALU.mult)
            i3 = ept(1)
            TT(i3[:], i2[:], is2[:], ALU.mult)
            dA = ept(1)
            TT(dA[:], detC2[:], i3[:], ALU.mult)
            h1 = ept(1)
            STT(h1[:], trA2[:], 0.5, dA[:], ALU.mult, ALU.add)
            D = ept(1)
            nc.vector.tensor_scalar_add(D[:], h1[:], -2.5)

            # x = D / sqrt(max(4 P2^3 - D^2, eps)) ;  phi = (pi/2 - atan(x))/3
            g1 = ept(1)
            TT(g1[:], P2c[:], P2c[:], ALU.mult)
            g2 = ept(1)
            TT(g2[:], g1[:], P2c[:], ALU.mult)
            g3 = ept(1)
            TT(g3[:], D[:], D[:], ALU.mult)
            G = ept(1)
            STT(G[:], g2[:], 4.0, g3[:], ALU.mult, ALU.subtract)
            Gc = ept(1)
            nc.vector.tensor_scalar_max(Gc[:], G[:], 1e-38)
            w_ = ept(1)
            nc.scalar.activation(w_[:], Gc[:], AFT.Sqrt)
            p_ = ept(1)
            nc.scalar.activation(p_[:], P2c[:], AFT.Sqrt)
            iw = ept(1)
            nc.vector.reciprocal(iw[:], w_[:])
            xx = ept(1)
            TT(xx[:], D[:], iw[:], ALU.mult)
            # atan with range reduction (ACT Arctan domain is [-pi/2, pi/2]):
            # atan(x) = sgn(x) * [ atan(m) + (|x|>1)*(pi/2 - 2*atan(m)) ],
            # m = min(|x|, 1/|x|)
            negx = ept(1)
            nc.vector.tensor_scalar_mul(negx[:], xx[:], -1.0)
            ax = ept(1)
            TT(ax[:], xx[:], negx[:], ALU.max)
            axc = ept(1)
            nc.vector.tensor_scalar_max(axc[:], ax[:], 1e-30)
            invx = ept(1)
            nc.vector.reciprocal(invx[:], axc[:])
            mn = ept(1)
            TT(mn[:], ax[:], invx[:], ALU.min)
            tt_ = ept(1)
            nc.scalar.activation(tt_[:], mn[:], AFT.Arctan)
            mbig = ept(1)
            TS(mbig[:], ax[:], 1.0, None, ALU.is_gt)
            v_ = ept(1)
            TS(v_[:], tt_[:], -2.0, float(np.pi / 2.0), ALU.mult, ALU.add)
            w2 = ept(1)
            TT(w2[:], v_[:], mbig[:], ALU.mult)
            atabs = ept(1)
            TT(atabs[:], tt_[:], w2[:], ALU.add)
            msgn = ept(1)
            TS(msgn[:], xx[:], 0.0, None, ALU.is_ge)
            sgn = ept(1)
            TS(sgn[:], msgn[:], 2.0, -1.0, ALU.mult, ALU.add)
            at = ept(1)
            TT(at[:], atabs[:], sgn[:], ALU.mult)
            phi = ept(1)
            TS(phi[:], at[:], -1.0 / 3.0, float(np.pi / 6.0), ALU.mult, ALU.add)
            th = ept(2)
            TT(th[:], cst, phi[:].broadcast_to([16, 2]), ALU.subtract)
            cc = ept(2)
            nc.scalar.activation(cc[:], th[:], AFT.Sin)

            # lam' = 1 + 2 p' cos(theta), lam = s2 * lam' ; lam1 >= lam2 >= lam3
            lam = ept(3)
            tp = ept(2)
            TT(tp[:], cc[:], p_[:].broadcast_to([16, 2]), ALU.mult)
            lam13 = lam[:, 0:3:2]
            TS(lam13, tp[:], 2.0, 1.0, ALU.mult, ALU.add)
            s13 = ept(1)
            nc.vector.tensor_reduce(s13[:], lam13, AX.X, ALU.add)
            TS(lam[:, 1:2], s13[:], -1.0, 3.0, ALU.mult, ALU.add)
            lamn = ept(3)
            nc.vector.tensor_scalar_mul(lamn[:], lam[:], s2[:, 0:1])
            lamc = ept(3)
            nc.vector.tensor_scalar_max(lamc[:], lamn[:], 0.0)
            sg = ept(3)
            nc.scalar.activation(sg[:], lamc[:], AFT.Sqrt)

            # sum_s = s0 + s1 + det(C)/(s0 s1);  rmsd = sqrt(relu(E0-2 sum_s)/n + 1e-8)
            pr = ept(1)
            TT(pr[:], sg[:, 0:1], sg[:, 1:2], ALU.mult)
            prc = ept(1)
            nc.vector.tensor_scalar_max(prc[:], pr[:], 1e-35)
            ipr = ept(1)
            nc.vector.reciprocal(ipr[:], prc[:])
            corr = ept(1)
            TT(corr[:], detC[:], ipr[:], ALU.mult)
            s01 = ept(1)
            TT(s01[:], sg[:, 0:1], sg[:, 1:2], ALU.add)
            sum_s = ept(1)
            TT(sum_s[:], s01[:], corr[:], ALU.add)
            t11 = ept(1)
            STT(t11[:], sum_s[:], -2.0, E0[:], ALU.mult, ALU.add)
            t12 = ept(1)
            nc.vector.tensor_scalar_max(t12[:], t11[:], 0.0)
            msd = ept(1)
            TT(msd[:], t12[:], rn[:], ALU.mult)
            msde = ept(1)
            TS(msde[:], msd[:], 1.0, 1e-8, ALU.mult, ALU.add)
            rmsd = ept(1)
            nc.scalar.activation(rmsd[:], msde[:], AFT.Sqrt)
            nc.sync.dma_start(out=o_d, in_=rmsd[:])

    nc.compile()
    return nc


def _host_aux(num_atoms_shard):
    """aux [128, AUXW] f32 for one core's 16 rows."""
    aux = np.zeros((128, AUXW), dtype=np.float32)
    p = np.arange(128)
    r = p // BLOCKS
    i = p % BLOCKS
    aux[p, r] = 1.0  # row selector
    n_of_p = num_atoms_shard[r].astype(np.float64)
    for t in range(NT):
        v = np.clip(n_of_p - CHUNK * i - APT * t, 0, APT)
        aux[:, COL_VT + t] = v.astype(np.float32)
    aux[0:ROWS, COL_N] = num_atoms_shard.astype(np.float32)
    aux[0:ROWS, COL_CST] = np.pi / 2.0
    aux[0:ROWS, COL_CST + 1] = -np.pi / 6.0
    return aux


def _host_mask(num_atoms_shard):
    import ml_dtypes

    m = (
        np.arange(MAX_ATOMS)[None, :] < np.asarray(num_atoms_shard)[:, None]
    )
    m3 = np.repeat(m, 3, axis=1)  # [ROWS, N3] interleaved xyz
    return np.ascontiguousarray(m3).astype(ml_dtypes.bfloat16)


def kernel(input, target, num_atoms):
    from concourse.bass_utils import run_bass_kernel_spmd

    if "nc" not in _state:
        _state["nc"] = _build()
    nc = _state["nc"]

    input = np.ascontiguousarray(np.asarray(input), dtype=np.float32)
    target = np.ascontiguousarray(np.asarray(target), dtype=np.float32)
    num_atoms = np.asarray(num_atoms)

    in_maps = []
    for c in range(NCORES):
        rs = slice(c * ROWS, (c + 1) * ROWS)
        in_maps.append(
            {
                "x": np.ascontiguousarray(input[rs]),
                "y": np.ascontiguousarray(target[rs]),
                "msk": _host_mask(np.asarray(num_atoms[rs])),
                "aux": _host_aux(np.asarray(num_atoms[rs])),
            }
        )

    res = run_bass_kernel_spmd(nc, in_maps, core_ids=list(range(NCORES)))
    out = np.concatenate([r["o"].reshape(ROWS) for r in res.results])
    return out.astype(np.float32)



# revision 7
# speedup vs baseline: 2.3590x; 2.3590x over previous
"""Trainium2 Bass kernel for batched masked-Kabsch RMSD (Coords2RMSD).

Contract: kernel(**inputs) takes FULL inputs (input [128, 49152] f32,
target [128, 49152] f32, num_atoms [128] i32) and returns the FULL
output [128] f32.  Shards batch rows across 8 NeuronCores (16 rows per
core), runs one SPMD Bass program, gathers.

Device algorithm (per core), v2 "PE-Gram" design:
  - Host ships a transposed, pre-masked tensor Z[a0, b, pl, r]:
    partition a0 = atom index within a 128-atom block b, plane
    pl in {x0,x1,x2,y0,y1,y2,mask}, r = batch row.  All 17 reduction
    stats (3x3 cross-covariance, column sums, squared norms) come from
    ONE accumulated PE Gram series: for each block b,
      G += Z[:,b,:112].T @ Z[:,b,:96]        (PSUM accumulate)
    The diagonal (r==r') blocks of G are the per-row stats; cross-row
    entries are garbage that a diag-mask multiply + segmented reduce
    discards.  A set of 7 tiny selector matmuls transposes the stats to
    [16 rows, 42 channels].
  - Epilogue (per row, 16 partitions): unnormalized trigonometric
    closed-form eigenvalues of C^T C, with asin/cos evaluated as DVE
    polynomials (no arctan/sin ACT tables; only the sqrt table is used,
    preloaded during the DMA phase).
"""

import os
import sys

import numpy as np

for _p in ("/opt/trn_rl_repo", "/root/.axon_site/_ro/trn_rl_repo"):
    if os.path.isdir(_p) and _p not in sys.path:
        sys.path.insert(0, _p)

B = 128
MAX_ATOMS = 16384
N3 = 3 * MAX_ATOMS          # 49152
NCORES = 8
ROWS = B // NCORES          # 16 rows per core
NBLK = MAX_ATOMS // 128     # 128 atom blocks of 128 atoms
PL = 7                      # planes: x0 x1 x2 y0 y1 y2 mask
PLR = 6                     # rhs planes (no mask)
MW = PL * ROWS              # 112 lhsT columns
NW = PLR * ROWS             # 96 rhs columns
NT = 4                      # DMA tiles along the block dim
BPT = NBLK // NT            # 32 blocks per DMA tile

# "fp16" or "fp8"
KMODE = os.environ.get("K_MODE", "fp16")

AUXW = 144
COL_DM = 112      # [112, 16] diag row mask
COL_N = 128       # per-row scalars live in rows 0:16
COL_RN = 129
COL_NRN = 130
COL_CPM = 131     # (2, -2)
COL_PIO = 133     # (0, pi/3)
COL_SRN = 135     # sqrt(1/n)
COL_EPS = 136     # 1e-8 (rmsd bias)

# asin(z) ~= z * sum_k ASIN_C[k] * z^(2k), least-squares fit on [-1,1]
ASIN_C = (0.947305116, 1.431119116, -8.268641827, 22.240540157,
          -25.607607179, 10.775150736)

_state = {}


def _build():
    import concourse.bacc as bacc
    import concourse.mybir as mybir
    import concourse.tile as tile

    dt = mybir.dt
    AFT = mybir.ActivationFunctionType
    ALU = mybir.AluOpType
    AX = mybir.AxisListType

    DT = dt.float16 if KMODE == "fp16" else dt.float8e4

    nc = bacc.Bacc("TRN2", target_bir_lowering=False, debug=False)

    z_d = nc.dram_tensor("z", [128, NBLK * MW], DT, kind="ExternalInput").ap()
    aux_d = nc.dram_tensor("aux", [112, AUXW], dt.float32, kind="ExternalInput").ap()
    o_d = nc.dram_tensor("o", [ROWS, 1], dt.float32, kind="ExternalOutput").ap()

    with tile.TileContext(nc) as tc:
        with (
            tc.tile_pool(name="data", bufs=1) as data_pool,
            tc.tile_pool(name="small", bufs=1) as small_pool,
            tc.tile_pool(name="ep", bufs=1) as ep_pool,
            tc.tile_pool(name="psum", bufs=1, space="PSUM") as psum_pool,
        ):
            # -------- bulk: DMA + accumulated PE Gram ------------------
            zt = []
            for t in range(NT):
                ztile = data_pool.tile([128, BPT * MW], DT, tag=f"z{t}")
                sl = slice(BPT * MW * t, BPT * MW * (t + 1))
                nc.sync.dma_start(out=ztile[:], in_=z_d[:, sl])
                zt.append(ztile)

            aux = small_pool.tile([112, AUXW], dt.float32)
            nc.sync.dma_start(out=aux[:], in_=aux_d)

            # preload the sqrt activation table while DMAs stream
            warm = small_pool.tile([1, 2], dt.float32)
            nc.vector.memset(warm[:], 1.0)
            nc.scalar.activation(warm[:, 1:2], warm[:, 0:1], AFT.Sqrt)

            G = psum_pool.tile([MW, NW], dt.float32)
            for b in range(NBLK):
                t, j = divmod(b, BPT)
                zb = zt[t][:].rearrange("p (j c) -> p j c", j=BPT)
                lhsT = zb[:, j, :]
                rhs = zb[:, j, 0:NW]
                nc.tensor.matmul(
                    G[:], lhsT, rhs,
                    start=(b == 0), stop=(b == NBLK - 1),
                )

            # -------- extract per-row stats from Gram diagonal ---------
            # R6[pl*16+r, pl'] = G[pl*16+r, pl'*16+r]
            Gm = ep_pool.tile([112, NW], dt.float32, name="Gm", tag="Gm")
            dmv = aux[:, COL_DM : COL_DM + 16]
            nc.vector.tensor_tensor(
                Gm[:].rearrange("p (c r) -> p c r", r=ROWS),
                G[:].rearrange("p (c r) -> p c r", r=ROWS),
                dmv.unsqueeze(1).broadcast_to([112, PLR, ROWS]),
                ALU.mult,
            )
            R6 = ep_pool.tile([112, PLR], dt.float32, name="R6", tag="R6")
            nc.vector.tensor_reduce(
                R6[:], Gm[:].rearrange("p (c r) -> p c r", r=ROWS), AX.X, ALU.add
            )
            # transpose stats to [16 rows, 42]: S42[r, 6*pl+pl']
            E2 = psum_pool.tile([ROWS, PL * PLR], dt.float32)
            for pl in range(PL):
                nc.tensor.matmul(
                    E2[:, PLR * pl : PLR * (pl + 1)],
                    aux[:, pl * 16 : (pl + 1) * 16],
                    R6[:],
                    start=True, stop=True,
                )
            S42 = ep_pool.tile([ROWS, PL * PLR], dt.float32, name="S42", tag="S42")
            nc.vector.tensor_scalar_mul(S42[:], E2[:], 1.0)

            # -------- epilogue ----------------------------------------
            _ep_ctr = [0]

            def ept(w):
                _ep_ctr[0] += 1
                nm = f"ep{_ep_ctr[0]}"
                return ep_pool.tile([ROWS, w], dt.float32, name=nm, tag=nm)

            TT = nc.vector.tensor_tensor
            STT = nc.vector.scalar_tensor_tensor
            TS = nc.vector.tensor_scalar

            nn = aux[0:ROWS, COL_N : COL_N + 1]
            rn = aux[0:ROWS, COL_RN : COL_RN + 1]
            nrn = aux[0:ROWS, COL_NRN : COL_NRN + 1]
            cpm = aux[0:ROWS, COL_CPM : COL_CPM + 2]
            pio = aux[0:ROWS, COL_PIO : COL_PIO + 2]
            srn = aux[0:ROWS, COL_SRN : COL_SRN + 1]
            eps8 = aux[0:ROWS, COL_EPS : COL_EPS + 1]

            # channel views of S42
            s6 = S42[:, 36:42]                 # sx (3), sy (3)
            sx = S42[:, 36:39]
            sy = S42[:, 39:42]
            M3 = S42[:, 3:21].rearrange("p (k l) -> p k l", l=PLR)[:, :, 0:3]
            diag6 = S42[:].rearrange("p (a b) -> p b a", b=PL)[:, 0:1, :]

            # E0 branch on ACT (parallel with DVE mainline):
            #   ssn = (|sx|^2+|sy|^2)/n  via Square(s * sqrt(1/n)) accum
            #   sxy = Sxx + Syy          via Identity accum over diag6
            ssn = ept(1)
            scr6 = ept(PLR)
            nc.scalar.activation(scr6[:], s6, AFT.Square, scale=srn,
                                 accum_out=ssn[:])
            sxy = ept(1)
            scr6b = ept(PLR)
            nc.scalar.activation(
                scr6b[:].rearrange("p (a b) -> p a b", a=1), diag6,
                AFT.Identity, accum_out=sxy[:],
            )
            E0 = ept(1)
            TT(E0[:], sxy[:], ssn[:], ALU.subtract)

            # C = M - sx sy^T / n
            O9 = ept(9)
            o3 = O9[:].rearrange("p (k l) -> p k l", l=3)
            TT(o3, sx.unsqueeze(2).broadcast_to([ROWS, 3, 3]),
               sy.unsqueeze(1).broadcast_to([ROWS, 3, 3]), ALU.mult)
            C9 = ept(9)
            STT(C9[:].rearrange("p (k l) -> p k l", l=3), o3,
                nrn[:, 0:1], M3, ALU.mult, ALU.add)

            # A = C^T C
            W27 = ept(27)
            w3 = W27[:].rearrange("p (i j a) -> p i j a", j=3, a=3)
            cu = C9[:].rearrange("p (a i) -> p i a", i=3).unsqueeze(2)
            cv = C9[:].rearrange("p (a j) -> p j a", j=3).unsqueeze(1)
            TT(w3, cu.broadcast_to([ROWS, 3, 3, 3]),
               cv.broadcast_to([ROWS, 3, 3, 3]), ALU.mult)
            A9 = ept(9)
            nc.vector.tensor_reduce(
                A9[:].rearrange("p (i j) -> p i j", j=3), w3, AX.X, ALU.add
            )
            # t = tr(A), q = tr(A^2) = sum A9^2
            t1 = ept(1)
            nc.vector.tensor_reduce(t1[:], A9[:, 0:9:4], AX.X, ALU.add)
            f2 = ept(9)
            q1 = ept(1)
            STT(f2[:], A9[:], 1.0, A9[:], ALU.mult, ALU.mult, accum_out=q1[:])
            t2 = ept(1)
            TT(t2[:], t1[:], t1[:], ALU.mult)
            t3 = ept(1)
            TT(t3[:], t2[:], t1[:], ALU.mult)

            # det(C) (signed)
            PA = ept(3)
            PB = ept(3)
            TT(PA[:, 0:1], C9[:, 4:5], C9[:, 8:9], ALU.mult)
            TT(PA[:, 1:2], C9[:, 5:6], C9[:, 6:7], ALU.mult)
            TT(PA[:, 2:3], C9[:, 3:4], C9[:, 7:8], ALU.mult)
            TT(PB[:, 0:1], C9[:, 5:6], C9[:, 7:8], ALU.mult)
            TT(PB[:, 1:2], C9[:, 3:4], C9[:, 8:9], ALU.mult)
            TT(PB[:, 2:3], C9[:, 4:5], C9[:, 6:7], ALU.mult)
            cof = ept(3)
            TT(cof[:], PA[:], PB[:], ALU.subtract)
            det3 = ept(3)
            detC = ept(1)
            STT(det3[:], C9[:, 0:3], 1.0, cof[:], ALU.mult, ALU.mult,
                accum_out=detC[:])

            # Du = det(A - (t/3) I) = detC^2 + t*q/6 - (5/54) t^3
            dA = ept(1)
            TT(dA[:], detC[:], detC[:], ALU.mult)
            tq = ept(1)
            TT(tq[:], t1[:], q1[:], ALU.mult)
            Du1 = ept(1)
            STT(Du1[:], tq[:], 1.0 / 6.0, dA[:], ALU.mult, ALU.add)
            Du = ept(1)
            STT(Du[:], t3[:], -5.0 / 54.0, Du1[:], ALU.mult, ALU.add)
            # P2u = (q - t^2/3)/6 = q/6 - t^2/18
            qq = ept(1)
            nc.vector.tensor_scalar_mul(qq[:], q1[:], 1.0 / 6.0)
            P2u = ept(1)
            STT(P2u[:], t2[:], -1.0 / 18.0, qq[:], ALU.mult, ALU.add)
            P2c = ept(1)
            nc.vector.tensor_scalar_max(P2c[:], P2u[:], 1e-20)

            # z = Du / (2 * P2c^1.5)
            r_ = ept(1)
            nc.scalar.activation(r_[:], P2c[:], AFT.Sqrt)
            w_ = ept(1)
            TT(w_[:], P2c[:], r_[:], ALU.mult)
            iw = ept(1)
            nc.vector.reciprocal(iw[:], w_[:])
            zz = ept(1)
            STT(zz[:], iw[:], 0.5, Du[:], ALU.mult, ALU.mult)

            # phi = (pi/2 - asin(z))/3 via odd polynomial
            uu = ept(1)
            TT(uu[:], zz[:], zz[:], ALU.mult)
            h = ept(1)
            TS(h[:], uu[:], ASIN_C[5], ASIN_C[4], ALU.mult, ALU.add)
            for c in (ASIN_C[3], ASIN_C[2], ASIN_C[1], ASIN_C[0]):
                h2 = ept(1)
                TT(h2[:], uu[:], h[:], ALU.mult)
                h3 = ept(1)
                nc.vector.tensor_scalar_add(h3[:], h2[:], c)
                h = h3
            asn = ept(1)
            TT(asn[:], zz[:], h[:], ALU.mult)
            phi = ept(1)
            TS(phi[:], asn[:], -1.0 / 3.0, float(np.pi / 6.0), ALU.mult, ALU.add)

            # cc = cos(th), th = {phi, phi - pi/3};  deg-4 poly
            th = ept(2)
            TT(th[:], phi[:].broadcast_to([ROWS, 2]), pio, ALU.subtract)
            u2 = ept(2)
            TT(u2[:], th[:], th[:], ALU.mult)
            hh = ept(2)
            TS(hh[:], u2[:], 1.0 / 24.0, -0.5, ALU.mult, ALU.add)
            uh = ept(2)
            TT(uh[:], u2[:], hh[:], ALU.mult)
            cc = ept(2)
            nc.vector.tensor_scalar_add(cc[:], uh[:], 1.0)

            # lam1 = t/3 + 2 r cos(phi); lam3 = t/3 - 2 r cos(pi/3 - phi)
            # (cos is even so cos(phi - pi/3) works); lam2 = t - lam1 - lam3
            di = ept(2)
            TT(di[:], r_[:].broadcast_to([ROWS, 2]), cpm, ALU.mult)
            tp = ept(2)
            TT(tp[:], cc[:], di[:], ALU.mult)
            b3 = ept(1)
            nc.vector.tensor_scalar_mul(b3[:], t1[:], 1.0 / 3.0)
            lam = ept(3)
            TT(lam[:, 0:3:2], tp[:], b3[:].broadcast_to([ROWS, 2]), ALU.add)
            s13 = ept(1)
            nc.vector.tensor_reduce(s13[:], lam[:, 0:3:2], AX.X, ALU.add)
            TT(lam[:, 1:2], t1[:], s13[:], ALU.subtract)
            lamc = ept(3)
            nc.vector.tensor_scalar_max(lamc[:], lam[:], 0.0)
            sg = ept(3)
            nc.scalar.activation(sg[:], lamc[:], AFT.Sqrt)

            # sum_s = s0 + s1 + det(C)/(s0 s1); rmsd = sqrt(relu(E0-2 sum_s)/n + 1e-8)
            pr = ept(1)
            TT(pr[:], sg[:, 0:1], sg[:, 1:2], ALU.mult)
            prc = ept(1)
            nc.vector.tensor_scalar_max(prc[:], pr[:], 1e-30)
            ipr = ept(1)
            nc.vector.reciprocal(ipr[:], prc[:])
            corr = ept(1)
            TT(corr[:], detC[:], ipr[:], ALU.mult)
            s01 = ept(1)
            TT(s01[:], sg[:, 0:1], sg[:, 1:2], ALU.add)
            sum_s = ept(1)
            TT(sum_s[:], s01[:], corr[:], ALU.add)
            t11 = ept(1)
            STT(t11[:], sum_s[:], -2.0, E0[:], ALU.mult, ALU.add)
            t12 = ept(1)
            nc.vector.tensor_scalar_max(t12[:], t11[:], 0.0)
            rmsd = ept(1)
            nc.scalar.activation(rmsd[:], t12[:], AFT.Sqrt, bias=eps8,
                                 scale=rn[:, 0:1])
            nc.sync.dma_start(out=o_d, in_=rmsd[:])

    nc.compile()
    return nc


def _np_dt():
    if KMODE == "fp16":
        return np.float16
    import ml_dtypes

    return ml_dtypes.float8_e4m3


def _host_z(x16, y16, n16):
    """Z [128, NBLK*112]: Z[a0, b, pl, r] = plane pl of row r atom b*128+a0."""
    m = (np.arange(MAX_ATOMS)[None, :] < n16[:, None])
    x3 = x16.reshape(ROWS, MAX_ATOMS, 3) * m[..., None]
    y3 = y16.reshape(ROWS, MAX_ATOMS, 3) * m[..., None]
    P = np.empty((PL, ROWS, MAX_ATOMS), np.float32)
    P[0:3] = np.moveaxis(x3, 2, 0)
    P[3:6] = np.moveaxis(y3, 2, 0)
    P[6] = m
    Z = P.reshape(PL, ROWS, NBLK, 128).transpose(3, 2, 0, 1)
    return np.ascontiguousarray(Z).reshape(128, NBLK * MW).astype(_np_dt())


def _host_aux(n16):
    aux = np.zeros((112, AUXW), dtype=np.float32)
    aux[:, 0:112] = np.eye(112, dtype=np.float32)
    p = np.arange(112)
    aux[p, COL_DM + (p % 16)] = 1.0
    nf = n16.astype(np.float64)
    aux[0:ROWS, COL_N] = nf
    aux[0:ROWS, COL_RN] = 1.0 / nf
    aux[0:ROWS, COL_NRN] = -1.0 / nf
    aux[0:ROWS, COL_CPM] = 2.0
    aux[0:ROWS, COL_CPM + 1] = -2.0
    aux[0:ROWS, COL_PIO] = 0.0
    aux[0:ROWS, COL_PIO + 1] = np.pi / 3.0
    aux[0:ROWS, COL_SRN] = np.sqrt(1.0 / nf)
    aux[0:ROWS, COL_EPS] = 1e-8
    return aux


def kernel(input, target, num_atoms):
    from concourse.bass_utils import run_bass_kernel_spmd

    if "nc" not in _state:
        _state["nc"] = _build()
    nc = _state["nc"]

    input = np.ascontiguousarray(np.asarray(input), dtype=np.float32)
    target = np.ascontiguousarray(np.asarray(target), dtype=np.float32)
    num_atoms = np.asarray(num_atoms)

    in_maps = []
    for c in range(NCORES):
        rs = slice(c * ROWS, (c + 1) * ROWS)
        n16 = np.asarray(num_atoms[rs])
        in_maps.append(
            {
                "z": _host_z(input[rs], target[rs], n16),
                "aux": _host_aux(n16),
            }
        )

    res = run_bass_kernel_spmd(nc, in_maps, core_ids=list(range(NCORES)))
    out = np.concatenate([r["o"].reshape(ROWS) for r in res.results])
    return out.astype(np.float32)


# revision 9
# speedup vs baseline: 3.1673x; 1.3426x over previous
"""Trainium2 Bass kernel for batched masked-Kabsch RMSD (Coords2RMSD).

Contract: kernel(**inputs) takes FULL inputs (input [128, 49152] f32,
target [128, 49152] f32, num_atoms [128] i32) and returns the FULL
output [128] f32.  Shards batch rows across 8 NeuronCores (16 rows per
core), runs one SPMD Bass program, gathers.

Device algorithm (per core), v2 "PE-Gram" design:
  - Host ships a transposed, pre-masked tensor Z[a0, b, pl, r]:
    partition a0 = atom index within a 128-atom block b, plane
    pl in {x0,x1,x2,y0,y1,y2,mask}, r = batch row.  All 17 reduction
    stats (3x3 cross-covariance, column sums, squared norms) come from
    ONE accumulated PE Gram series: for each block b,
      G += Z[:,b,:112].T @ Z[:,b,:96]        (PSUM accumulate)
    The diagonal (r==r') blocks of G are the per-row stats; cross-row
    entries are garbage that a diag-mask multiply + segmented reduce
    discards.  A set of 7 tiny selector matmuls transposes the stats to
    [16 rows, 42 channels].
  - Epilogue (per row, 16 partitions): unnormalized trigonometric
    closed-form eigenvalues of C^T C, with asin/cos evaluated as DVE
    polynomials (no arctan/sin ACT tables; only the sqrt table is used,
    preloaded during the DMA phase).
"""

import os
import sys

import numpy as np

for _p in ("/opt/trn_rl_repo", "/root/.axon_site/_ro/trn_rl_repo"):
    if os.path.isdir(_p) and _p not in sys.path:
        sys.path.insert(0, _p)

B = 128
MAX_ATOMS = 16384
N3 = 3 * MAX_ATOMS          # 49152
NCORES = 8
ROWS = B // NCORES          # 16 rows per core
NBLK = MAX_ATOMS // 128     # 128 atom blocks of 128 atoms
PL = 7                      # planes: x0 x1 x2 y0 y1 y2 mask
PLR = 6                     # rhs planes (no mask)
MW = PL * ROWS              # 112 lhsT columns
NW = PLR * ROWS             # 96 rhs columns
NT = 4                      # DMA tiles along the block dim
BPT = NBLK // NT            # 32 blocks per DMA tile

# "fp16" or "fp8" (fp8 uses DoubleRow matmuls: 2 k-tiles per pass)
KMODE = os.environ.get("K_MODE", "fp8")

AUXW = 144
COL_DM = 112      # [112, 16] diag row mask
COL_N = 128       # per-row scalars live in rows 0:16
COL_RN = 129
COL_NRN = 130
COL_CPM = 131     # (2, -2)
COL_PIO = 133     # (0, pi/3)
COL_SRN = 135     # sqrt(1/n)
COL_EPS = 136     # 1e-8 (rmsd bias)

# asin(z) ~= z * sum_k ASIN_C[k] * z^(2k), least-squares fit on [-1,1]
ASIN_C = (0.947305116, 1.431119116, -8.268641827, 22.240540157,
          -25.607607179, 10.775150736)

_state = {}


def _build():
    import concourse.bacc as bacc
    import concourse.mybir as mybir
    import concourse.tile as tile

    dt = mybir.dt
    AFT = mybir.ActivationFunctionType
    ALU = mybir.AluOpType
    AX = mybir.AxisListType

    DT = dt.float16 if KMODE == "fp16" else dt.float8e4

    nc = bacc.Bacc("TRN2", target_bir_lowering=False, debug=False)

    z_d = nc.dram_tensor("z", [128, NBLK * MW], DT, kind="ExternalInput").ap()
    aux_d = nc.dram_tensor("aux", [112, AUXW], dt.float32, kind="ExternalInput").ap()
    o_d = nc.dram_tensor("o", [ROWS, 1], dt.float32, kind="ExternalOutput").ap()

    with tile.TileContext(nc) as tc:
        with (
            tc.tile_pool(name="data", bufs=1) as data_pool,
            tc.tile_pool(name="small", bufs=1) as small_pool,
            tc.tile_pool(name="ep", bufs=1) as ep_pool,
            tc.tile_pool(name="psum", bufs=1, space="PSUM") as psum_pool,
        ):
            # -------- bulk: DMA + accumulated PE Gram ------------------
            zt = []
            for t in range(NT):
                ztile = data_pool.tile([128, BPT * MW], DT, tag=f"z{t}")
                sl = slice(BPT * MW * t, BPT * MW * (t + 1))
                nc.sync.dma_start(out=ztile[:], in_=z_d[:, sl])
                zt.append(ztile)

            aux = small_pool.tile([112, AUXW], dt.float32)
            nc.sync.dma_start(out=aux[:], in_=aux_d)

            # preload the sqrt activation table while DMAs stream
            warm = small_pool.tile([1, 2], dt.float32)
            nc.vector.memset(warm[:], 1.0)
            nc.scalar.activation(warm[:, 1:2], warm[:, 0:1], AFT.Sqrt)

            G = psum_pool.tile([MW, NW], dt.float32)
            if KMODE == "fp8":
                PAIRS = NBLK // 2
                PPT = PAIRS // NT
                for b2 in range(PAIRS):
                    t, j2 = divmod(b2, PPT)
                    zb = zt[t][:].rearrange(
                        "p (j two c) -> p j two c", j=PPT, two=2
                    )
                    nc.tensor.matmul(
                        G[:], zb[:, j2, :, :], zb[:, j2, :, 0:NW],
                        start=(b2 == 0), stop=(b2 == PAIRS - 1),
                        perf_mode=mybir.MatmulPerfMode.DoubleRow,
                    )
            else:
                for b in range(NBLK):
                    t, j = divmod(b, BPT)
                    zb = zt[t][:].rearrange("p (j c) -> p j c", j=BPT)
                    nc.tensor.matmul(
                        G[:], zb[:, j, :], zb[:, j, 0:NW],
                        start=(b == 0), stop=(b == NBLK - 1),
                    )

            # -------- extract per-row stats from Gram diagonal ---------
            # R6[pl*16+r, pl'] = G[pl*16+r, pl'*16+r]
            Gm = ep_pool.tile([112, NW], dt.float32, name="Gm", tag="Gm")
            dmv = aux[:, COL_DM : COL_DM + 16]
            nc.vector.tensor_tensor(
                Gm[:].rearrange("p (c r) -> p c r", r=ROWS),
                G[:].rearrange("p (c r) -> p c r", r=ROWS),
                dmv.unsqueeze(1).broadcast_to([112, PLR, ROWS]),
                ALU.mult,
            )
            R6 = ep_pool.tile([112, PLR], dt.float32, name="R6", tag="R6")
            nc.vector.tensor_reduce(
                R6[:], Gm[:].rearrange("p (c r) -> p c r", r=ROWS), AX.X, ALU.add
            )
            # transpose stats to [16 rows, 42]: S42[r, 6*pl+pl']
            E2 = psum_pool.tile([ROWS, PL * PLR], dt.float32)
            for pl in range(PL):
                nc.tensor.matmul(
                    E2[:, PLR * pl : PLR * (pl + 1)],
                    aux[:, pl * 16 : (pl + 1) * 16],
                    R6[:],
                    start=True, stop=True,
                )
            S42 = ep_pool.tile([ROWS, PL * PLR], dt.float32, name="S42", tag="S42")
            nc.vector.tensor_scalar_mul(S42[:], E2[:], 1.0)

            # -------- epilogue ----------------------------------------
            _ep_ctr = [0]

            def ept(w):
                _ep_ctr[0] += 1
                nm = f"ep{_ep_ctr[0]}"
                return ep_pool.tile([ROWS, w], dt.float32, name=nm, tag=nm)

            TT = nc.vector.tensor_tensor
            STT = nc.vector.scalar_tensor_tensor
            TS = nc.vector.tensor_scalar

            nn = aux[0:ROWS, COL_N : COL_N + 1]
            rn = aux[0:ROWS, COL_RN : COL_RN + 1]
            nrn = aux[0:ROWS, COL_NRN : COL_NRN + 1]
            cpm = aux[0:ROWS, COL_CPM : COL_CPM + 2]
            pio = aux[0:ROWS, COL_PIO : COL_PIO + 2]
            srn = aux[0:ROWS, COL_SRN : COL_SRN + 1]
            eps8 = aux[0:ROWS, COL_EPS : COL_EPS + 1]

            # channel views of S42
            s6 = S42[:, 36:42]                 # sx (3), sy (3)
            sx = S42[:, 36:39]
            sy = S42[:, 39:42]
            M3 = S42[:, 3:21].rearrange("p (k l) -> p k l", l=PLR)[:, :, 0:3]
            diag6 = S42[:].rearrange("p (a b) -> p b a", b=PL)[:, 0:1, :]

            # E0 branch on ACT (parallel with DVE mainline):
            #   ssn = (|sx|^2+|sy|^2)/n  via Square(s * sqrt(1/n)) accum
            #   sxy = Sxx + Syy          via Identity accum over diag6
            ssn = ept(1)
            scr6 = ept(PLR)
            nc.scalar.activation(scr6[:], s6, AFT.Square, scale=srn,
                                 accum_out=ssn[:])
            sxy = ept(1)
            scr6b = ept(PLR)
            nc.scalar.activation(
                scr6b[:].rearrange("p (a b) -> p a b", a=1), diag6,
                AFT.Identity, accum_out=sxy[:],
            )
            E0 = ept(1)
            TT(E0[:], sxy[:], ssn[:], ALU.subtract)

            # C = M - sx sy^T / n
            O9 = ept(9)
            o3 = O9[:].rearrange("p (k l) -> p k l", l=3)
            TT(o3, sx.unsqueeze(2).broadcast_to([ROWS, 3, 3]),
               sy.unsqueeze(1).broadcast_to([ROWS, 3, 3]), ALU.mult)
            C9 = ept(9)
            STT(C9[:].rearrange("p (k l) -> p k l", l=3), o3,
                nrn[:, 0:1], M3, ALU.mult, ALU.add)

            # A = C^T C
            W27 = ept(27)
            w3 = W27[:].rearrange("p (i j a) -> p i j a", j=3, a=3)
            cu = C9[:].rearrange("p (a i) -> p i a", i=3).unsqueeze(2)
            cv = C9[:].rearrange("p (a j) -> p j a", j=3).unsqueeze(1)
            TT(w3, cu.broadcast_to([ROWS, 3, 3, 3]),
               cv.broadcast_to([ROWS, 3, 3, 3]), ALU.mult)
            A9 = ept(9)
            nc.vector.tensor_reduce(
                A9[:].rearrange("p (i j) -> p i j", j=3), w3, AX.X, ALU.add
            )
            # t = tr(A), q = tr(A^2) = sum A9^2
            t1 = ept(1)
            nc.vector.tensor_reduce(t1[:], A9[:, 0:9:4], AX.X, ALU.add)
            f2 = ept(9)
            q1 = ept(1)
            STT(f2[:], A9[:], 1.0, A9[:], ALU.mult, ALU.mult, accum_out=q1[:])
            t2 = ept(1)
            TT(t2[:], t1[:], t1[:], ALU.mult)
            t3 = ept(1)
            TT(t3[:], t2[:], t1[:], ALU.mult)

            # det(C) (signed)
            PA = ept(3)
            PB = ept(3)
            TT(PA[:, 0:1], C9[:, 4:5], C9[:, 8:9], ALU.mult)
            TT(PA[:, 1:2], C9[:, 5:6], C9[:, 6:7], ALU.mult)
            TT(PA[:, 2:3], C9[:, 3:4], C9[:, 7:8], ALU.mult)
            TT(PB[:, 0:1], C9[:, 5:6], C9[:, 7:8], ALU.mult)
            TT(PB[:, 1:2], C9[:, 3:4], C9[:, 8:9], ALU.mult)
            TT(PB[:, 2:3], C9[:, 4:5], C9[:, 6:7], ALU.mult)
            cof = ept(3)
            TT(cof[:], PA[:], PB[:], ALU.subtract)
            det3 = ept(3)
            detC = ept(1)
            STT(det3[:], C9[:, 0:3], 1.0, cof[:], ALU.mult, ALU.mult,
                accum_out=detC[:])

            # Du = det(A - (t/3) I) = detC^2 + t*q/6 - (5/54) t^3
            dA = ept(1)
            TT(dA[:], detC[:], detC[:], ALU.mult)
            tq = ept(1)
            TT(tq[:], t1[:], q1[:], ALU.mult)
            Du1 = ept(1)
            STT(Du1[:], tq[:], 1.0 / 6.0, dA[:], ALU.mult, ALU.add)
            Du = ept(1)
            STT(Du[:], t3[:], -5.0 / 54.0, Du1[:], ALU.mult, ALU.add)
            # P2u = (q - t^2/3)/6 = q/6 - t^2/18
            qq = ept(1)
            nc.vector.tensor_scalar_mul(qq[:], q1[:], 1.0 / 6.0)
            P2u = ept(1)
            STT(P2u[:], t2[:], -1.0 / 18.0, qq[:], ALU.mult, ALU.add)
            P2c = ept(1)
            nc.vector.tensor_scalar_max(P2c[:], P2u[:], 1e-20)

            # z = Du / (2 * P2c^1.5)
            r_ = ept(1)
            nc.scalar.activation(r_[:], P2c[:], AFT.Sqrt)
            w_ = ept(1)
            TT(w_[:], P2c[:], r_[:], ALU.mult)
            iw = ept(1)
            nc.vector.reciprocal(iw[:], w_[:])
            zz = ept(1)
            STT(zz[:], iw[:], 0.5, Du[:], ALU.mult, ALU.mult)

            # phi = (pi/2 - asin(z))/3 via odd polynomial
            uu = ept(1)
            TT(uu[:], zz[:], zz[:], ALU.mult)
            h = ept(1)
            TS(h[:], uu[:], ASIN_C[5], ASIN_C[4], ALU.mult, ALU.add)
            for c in (ASIN_C[3], ASIN_C[2], ASIN_C[1], ASIN_C[0]):
                h2 = ept(1)
                TT(h2[:], uu[:], h[:], ALU.mult)
                h3 = ept(1)
                nc.vector.tensor_scalar_add(h3[:], h2[:], c)
                h = h3
            asn = ept(1)
            TT(asn[:], zz[:], h[:], ALU.mult)
            phi = ept(1)
            TS(phi[:], asn[:], -1.0 / 3.0, float(np.pi / 6.0), ALU.mult, ALU.add)

            # cc = cos(th), th = {phi, phi - pi/3};  deg-4 poly
            th = ept(2)
            TT(th[:], phi[:].broadcast_to([ROWS, 2]), pio, ALU.subtract)
            u2 = ept(2)
            TT(u2[:], th[:], th[:], ALU.mult)
            hh = ept(2)
            TS(hh[:], u2[:], 1.0 / 24.0, -0.5, ALU.mult, ALU.add)
            uh = ept(2)
            TT(uh[:], u2[:], hh[:], ALU.mult)
            cc = ept(2)
            nc.vector.tensor_scalar_add(cc[:], uh[:], 1.0)

            # lam1 = t/3 + 2 r cos(phi); lam3 = t/3 - 2 r cos(pi/3 - phi)
            # (cos is even so cos(phi - pi/3) works); lam2 = t - lam1 - lam3
            di = ept(2)
            TT(di[:], r_[:].broadcast_to([ROWS, 2]), cpm, ALU.mult)
            tp = ept(2)
            TT(tp[:], cc[:], di[:], ALU.mult)
            b3 = ept(1)
            nc.vector.tensor_scalar_mul(b3[:], t1[:], 1.0 / 3.0)
            lam = ept(3)
            TT(lam[:, 0:3:2], tp[:], b3[:].broadcast_to([ROWS, 2]), ALU.add)
            s13 = ept(1)
            nc.vector.tensor_reduce(s13[:], lam[:, 0:3:2], AX.X, ALU.add)
            TT(lam[:, 1:2], t1[:], s13[:], ALU.subtract)
            lamc = ept(3)
            nc.vector.tensor_scalar_max(lamc[:], lam[:], 0.0)
            sg = ept(3)
            nc.scalar.activation(sg[:], lamc[:], AFT.Sqrt)

            # sum_s = s0 + s1 + det(C)/(s0 s1); rmsd = sqrt(relu(E0-2 sum_s)/n + 1e-8)
            pr = ept(1)
            TT(pr[:], sg[:, 0:1], sg[:, 1:2], ALU.mult)
            prc = ept(1)
            nc.vector.tensor_scalar_max(prc[:], pr[:], 1e-30)
            ipr = ept(1)
            nc.vector.reciprocal(ipr[:], prc[:])
            corr = ept(1)
            TT(corr[:], detC[:], ipr[:], ALU.mult)
            s01 = ept(1)
            TT(s01[:], sg[:, 0:1], sg[:, 1:2], ALU.add)
            sum_s = ept(1)
            TT(sum_s[:], s01[:], corr[:], ALU.add)
            t11 = ept(1)
            STT(t11[:], sum_s[:], -2.0, E0[:], ALU.mult, ALU.add)
            t12 = ept(1)
            nc.vector.tensor_scalar_max(t12[:], t11[:], 0.0)
            rmsd = ept(1)
            nc.scalar.activation(rmsd[:], t12[:], AFT.Sqrt, bias=eps8,
                                 scale=rn[:, 0:1])
            nc.sync.dma_start(out=o_d, in_=rmsd[:])

    nc.compile()
    return nc


def _np_dt():
    if KMODE == "fp16":
        return np.float16
    import ml_dtypes

    return ml_dtypes.float8_e4m3


def _host_z(x16, y16, n16):
    """Z [128, NBLK*112]: Z[a0, b, pl, r] = plane pl of row r atom b*128+a0."""
    m = (np.arange(MAX_ATOMS)[None, :] < n16[:, None])
    x3 = x16.reshape(ROWS, MAX_ATOMS, 3) * m[..., None]
    y3 = y16.reshape(ROWS, MAX_ATOMS, 3) * m[..., None]
    P = np.empty((PL, ROWS, MAX_ATOMS), np.float32)
    P[0:3] = np.moveaxis(x3, 2, 0)
    P[3:6] = np.moveaxis(y3, 2, 0)
    P[6] = m
    Z = P.reshape(PL, ROWS, NBLK, 128).transpose(3, 2, 0, 1)
    return np.ascontiguousarray(Z).reshape(128, NBLK * MW).astype(_np_dt())


def _host_aux(n16):
    aux = np.zeros((112, AUXW), dtype=np.float32)
    aux[:, 0:112] = np.eye(112, dtype=np.float32)
    p = np.arange(112)
    aux[p, COL_DM + (p % 16)] = 1.0
    nf = n16.astype(np.float64)
    aux[0:ROWS, COL_N] = nf
    aux[0:ROWS, COL_RN] = 1.0 / nf
    aux[0:ROWS, COL_NRN] = -1.0 / nf
    aux[0:ROWS, COL_CPM] = 2.0
    aux[0:ROWS, COL_CPM + 1] = -2.0
    aux[0:ROWS, COL_PIO] = 0.0
    aux[0:ROWS, COL_PIO + 1] = np.pi / 3.0
    aux[0:ROWS, COL_SRN] = np.sqrt(1.0 / nf)
    aux[0:ROWS, COL_EPS] = 1e-8
    return aux


def kernel(input, target, num_atoms):
    from concourse.bass_utils import run_bass_kernel_spmd

    if "nc" not in _state:
        _state["nc"] = _build()
    nc = _state["nc"]

    input = np.ascontiguousarray(np.asarray(input), dtype=np.float32)
    target = np.ascontiguousarray(np.asarray(target), dtype=np.float32)
    num_atoms = np.asarray(num_atoms)

    in_maps = []
    for c in range(NCORES):
        rs = slice(c * ROWS, (c + 1) * ROWS)
        n16 = np.asarray(num_atoms[rs])
        in_maps.append(
            {
                "z": _host_z(input[rs], target[rs], n16),
                "aux": _host_aux(n16),
            }
        )

    res = run_bass_kernel_spmd(nc, in_maps, core_ids=list(range(NCORES)))
    out = np.concatenate([r["o"].reshape(ROWS) for r in res.results])
    return out.astype(np.float32)


# revision 11
# speedup vs baseline: 3.4519x; 1.0899x over previous
"""Trainium2 Bass kernel for batched masked-Kabsch RMSD (Coords2RMSD).

Contract: kernel(**inputs) takes FULL inputs (input [128, 49152] f32,
target [128, 49152] f32, num_atoms [128] i32) and returns the FULL
output [128] f32.  Shards batch rows across 8 NeuronCores (16 rows per
core), runs one SPMD Bass program, gathers.

Device algorithm (per core), v2 "PE-Gram" design:
  - Host ships a transposed, pre-masked tensor Z[a0, b, pl, r]:
    partition a0 = atom index within a 128-atom block b, plane
    pl in {x0,x1,x2,y0,y1,y2,mask}, r = batch row.  All 17 reduction
    stats (3x3 cross-covariance, column sums, squared norms) come from
    ONE accumulated PE Gram series: for each block b,
      G += Z[:,b,:112].T @ Z[:,b,:96]        (PSUM accumulate)
    The diagonal (r==r') blocks of G are the per-row stats; cross-row
    entries are garbage that a diag-mask multiply + segmented reduce
    discards.  A set of 7 tiny selector matmuls transposes the stats to
    [16 rows, 42 channels].
  - Epilogue (per row, 16 partitions): unnormalized trigonometric
    closed-form eigenvalues of C^T C, with asin/cos evaluated as DVE
    polynomials (no arctan/sin ACT tables; only the sqrt table is used,
    preloaded during the DMA phase).
"""

import os
import sys

import numpy as np

for _p in ("/opt/trn_rl_repo", "/root/.axon_site/_ro/trn_rl_repo"):
    if os.path.isdir(_p) and _p not in sys.path:
        sys.path.insert(0, _p)

B = 128
MAX_ATOMS = 16384
N3 = 3 * MAX_ATOMS          # 49152
NCORES = 8
ROWS = B // NCORES          # 16 rows per core
NBLK = MAX_ATOMS // 128     # 128 atom blocks of 128 atoms
PL = 7                      # planes: x0 x1 x2 y0 y1 y2 mask
PLR = 6                     # rhs planes (no mask)
MW = PL * ROWS              # 112 lhsT columns
NW = PLR * ROWS             # 96 rhs columns
NT = 4                      # DMA tiles along the block dim
BPT = NBLK // NT            # 32 blocks per DMA tile

# "fp16" or "fp8" (fp8 uses DoubleRow matmuls: 2 k-tiles per pass)
KMODE = os.environ.get("K_MODE", "fp8")

AUXW = 144
COL_DM = 112      # [112, 16] diag row mask
COL_N = 128       # per-row scalars live in rows 0:16
COL_RN = 129
COL_NRN = 130
COL_CPM = 131     # (2, -2)
COL_PIO = 133     # (0, pi/3)
COL_SRN = 135     # sqrt(1/n)
COL_EPS = 136     # 1e-8 (rmsd bias)

# sin(asin(z)/3)  ~= z * (SA[0] + SA[1] u + SA[2] u^2), u = z^2, on [-1,1]
SA = (0.363286354, -0.129956059, 0.236283775)
# sqrt(3)*cos(asin(z)/3) ~= CA[0] + CA[1] u + CA[2] u^2
CA = (1.725367531, -0.003965617, -0.185061429)

_state = {}


def _build():
    import concourse.bacc as bacc
    import concourse.mybir as mybir
    import concourse.tile as tile

    dt = mybir.dt
    AFT = mybir.ActivationFunctionType
    ALU = mybir.AluOpType
    AX = mybir.AxisListType

    DT = dt.float16 if KMODE == "fp16" else dt.float8e4

    nc = bacc.Bacc("TRN2", target_bir_lowering=False, debug=False)

    z_d = nc.dram_tensor("z", [128, NBLK * MW], DT, kind="ExternalInput").ap()
    aux_d = nc.dram_tensor("aux", [112, AUXW], dt.float32, kind="ExternalInput").ap()
    o_d = nc.dram_tensor("o", [ROWS, 1], dt.float32, kind="ExternalOutput").ap()

    with tile.TileContext(nc) as tc:
        with (
            tc.tile_pool(name="data", bufs=1) as data_pool,
            tc.tile_pool(name="small", bufs=1) as small_pool,
            tc.tile_pool(name="ep", bufs=1) as ep_pool,
            tc.tile_pool(name="psum", bufs=1, space="PSUM") as psum_pool,
        ):
            # -------- bulk: DMA + accumulated PE Gram ------------------
            zt = []
            for t in range(NT):
                ztile = data_pool.tile([128, BPT * MW], DT, tag=f"z{t}")
                sl = slice(BPT * MW * t, BPT * MW * (t + 1))
                nc.sync.dma_start(out=ztile[:], in_=z_d[:, sl])
                zt.append(ztile)

            aux = small_pool.tile([112, AUXW], dt.float32)
            nc.sync.dma_start(out=aux[:], in_=aux_d)

            # preload the sqrt activation table while DMAs stream
            warm = small_pool.tile([1, 2], dt.float32)
            nc.vector.memset(warm[:], 1.0)
            nc.scalar.activation(warm[:, 1:2], warm[:, 0:1], AFT.Sqrt)

            G = psum_pool.tile([MW, NW], dt.float32)
            if KMODE == "fp8":
                PAIRS = NBLK // 2
                PPT = PAIRS // NT
                for b2 in range(PAIRS):
                    t, j2 = divmod(b2, PPT)
                    zb = zt[t][:].rearrange(
                        "p (j two c) -> p j two c", j=PPT, two=2
                    )
                    nc.tensor.matmul(
                        G[:], zb[:, j2, :, :], zb[:, j2, :, 0:NW],
                        start=(b2 == 0), stop=(b2 == PAIRS - 1),
                        perf_mode=mybir.MatmulPerfMode.DoubleRow,
                    )
            else:
                for b in range(NBLK):
                    t, j = divmod(b, BPT)
                    zb = zt[t][:].rearrange("p (j c) -> p j c", j=BPT)
                    nc.tensor.matmul(
                        G[:], zb[:, j, :], zb[:, j, 0:NW],
                        start=(b == 0), stop=(b == NBLK - 1),
                    )

            # -------- extract per-row stats from Gram diagonal ---------
            # R6[pl*16+r, pl'] = G[pl*16+r, pl'*16+r]
            Gm = ep_pool.tile([112, NW], dt.float32, name="Gm", tag="Gm")
            dmv = aux[:, COL_DM : COL_DM + 16]
            nc.vector.tensor_tensor(
                Gm[:].rearrange("p (c r) -> p c r", r=ROWS),
                G[:].rearrange("p (c r) -> p c r", r=ROWS),
                dmv.unsqueeze(1).broadcast_to([112, PLR, ROWS]),
                ALU.mult,
            )
            R6 = ep_pool.tile([112, PLR], dt.float32, name="R6", tag="R6")
            nc.vector.tensor_reduce(
                R6[:], Gm[:].rearrange("p (c r) -> p c r", r=ROWS), AX.X, ALU.add
            )
            # transpose stats to [16 rows, 42]: S42[r, 6*pl+pl']
            E2 = psum_pool.tile([ROWS, PL * PLR], dt.float32)
            for pl in range(PL):
                nc.tensor.matmul(
                    E2[:, PLR * pl : PLR * (pl + 1)],
                    aux[:, pl * 16 : (pl + 1) * 16],
                    R6[:],
                    start=True, stop=True,
                )
            S42 = ep_pool.tile([ROWS, PL * PLR], dt.float32, name="S42", tag="S42")
            nc.vector.tensor_scalar_mul(S42[:], E2[:], 1.0)

            # -------- epilogue ----------------------------------------
            _ep_ctr = [0]

            def ept(w):
                _ep_ctr[0] += 1
                nm = f"ep{_ep_ctr[0]}"
                return ep_pool.tile([ROWS, w], dt.float32, name=nm, tag=nm)

            TT = nc.vector.tensor_tensor
            STT = nc.vector.scalar_tensor_tensor
            TS = nc.vector.tensor_scalar

            nn = aux[0:ROWS, COL_N : COL_N + 1]
            rn = aux[0:ROWS, COL_RN : COL_RN + 1]
            nrn = aux[0:ROWS, COL_NRN : COL_NRN + 1]
            cpm = aux[0:ROWS, COL_CPM : COL_CPM + 2]
            pio = aux[0:ROWS, COL_PIO : COL_PIO + 2]
            srn = aux[0:ROWS, COL_SRN : COL_SRN + 1]
            eps8 = aux[0:ROWS, COL_EPS : COL_EPS + 1]

            # channel views of S42
            s6 = S42[:, 36:42]                 # sx (3), sy (3)
            sx = S42[:, 36:39]
            sy = S42[:, 39:42]
            M3 = S42[:, 3:21].rearrange("p (k l) -> p k l", l=PLR)[:, :, 0:3]
            diag6 = S42[:].rearrange("p (a b) -> p b a", b=PL)[:, 0:1, :]

            # E0 branch on ACT (parallel with DVE mainline):
            #   ssn = (|sx|^2+|sy|^2)/n  via Square(s * sqrt(1/n)) accum
            #   sxy = Sxx + Syy          via Identity accum over diag6
            ssn = ept(1)
            scr6 = ept(PLR)
            nc.scalar.activation(scr6[:], s6, AFT.Square, scale=srn,
                                 accum_out=ssn[:])
            sxy = ept(1)
            scr6b = ept(PLR)
            nc.scalar.activation(
                scr6b[:].rearrange("p (a b) -> p a b", a=1), diag6,
                AFT.Identity, accum_out=sxy[:],
            )
            E0 = ept(1)
            TT(E0[:], sxy[:], ssn[:], ALU.subtract)

            # C = M - sx sy^T / n
            O9 = ept(9)
            o3 = O9[:].rearrange("p (k l) -> p k l", l=3)
            TT(o3, sx.unsqueeze(2).broadcast_to([ROWS, 3, 3]),
               sy.unsqueeze(1).broadcast_to([ROWS, 3, 3]), ALU.mult)
            C9 = ept(9)
            STT(C9[:].rearrange("p (k l) -> p k l", l=3), o3,
                nrn[:, 0:1], M3, ALU.mult, ALU.add)

            # det(C): rows 1,2 duplicated so cofactors are contiguous slices
            D6 = ept(6)
            E6 = ept(6)
            nc.vector.tensor_scalar_mul(
                D6[:].rearrange("p (a b) -> p a b", a=2),
                C9[:, 3:6].unsqueeze(1).broadcast_to([ROWS, 2, 3]), 1.0)
            nc.vector.tensor_scalar_mul(
                E6[:].rearrange("p (a b) -> p a b", a=2),
                C9[:, 6:9].unsqueeze(1).broadcast_to([ROWS, 2, 3]), 1.0)

            # A = C^T C
            W27 = ept(27)
            w3 = W27[:].rearrange("p (i j a) -> p i j a", j=3, a=3)
            cu = C9[:].rearrange("p (a i) -> p i a", i=3).unsqueeze(2)
            cv = C9[:].rearrange("p (a j) -> p j a", j=3).unsqueeze(1)
            TT(w3, cu.broadcast_to([ROWS, 3, 3, 3]),
               cv.broadcast_to([ROWS, 3, 3, 3]), ALU.mult)
            A9 = ept(9)
            nc.vector.tensor_reduce(
                A9[:].rearrange("p (i j) -> p i j", j=3), w3, AX.X, ALU.add
            )

            cofA = ept(3)
            cofB = ept(3)
            TT(cofA[:], D6[:, 1:4], E6[:, 2:5], ALU.mult)
            TT(cofB[:], D6[:, 2:5], E6[:, 1:4], ALU.mult)
            cof = ept(3)
            TT(cof[:], cofA[:], cofB[:], ALU.subtract)
            det3 = ept(3)
            detC = ept(1)
            STT(det3[:], C9[:, 0:3], 1.0, cof[:], ALU.mult, ALU.mult,
                accum_out=detC[:])

            # t = tr(A), q = tr(A^2) = sum A9^2
            t1 = ept(1)
            nc.vector.tensor_reduce(t1[:], A9[:, 0:9:4], AX.X, ALU.add)
            f2 = ept(9)
            q1 = ept(1)
            STT(f2[:], A9[:], 1.0, A9[:], ALU.mult, ALU.mult, accum_out=q1[:])
            t2 = ept(1)
            TT(t2[:], t1[:], t1[:], ALU.mult)
            t3 = ept(1)
            TT(t3[:], t2[:], t1[:], ALU.mult)
            dA = ept(1)
            TT(dA[:], detC[:], detC[:], ALU.mult)
            # sign(detC) -> {-1, +1} (for the reflection term d*s_min)
            sgn = ept(1)
            TS(sgn[:], detC[:], 0.0, None, ALU.is_ge)
            sgn2 = ept(1)
            TS(sgn2[:], sgn[:], 2.0, -1.0, ALU.mult, ALU.add)

            # Du = det(A - (t/3) I) = detC^2 + t*q/6 - (5/54) t^3
            tq = ept(1)
            TT(tq[:], t1[:], q1[:], ALU.mult)
            Du1 = ept(1)
            STT(Du1[:], tq[:], 1.0 / 6.0, dA[:], ALU.mult, ALU.add)
            Du = ept(1)
            STT(Du[:], t3[:], -5.0 / 54.0, Du1[:], ALU.mult, ALU.add)
            # P2c = max((q - t^2/3)/6, eps)
            j1 = ept(1)
            STT(j1[:], t2[:], -1.0 / 3.0, q1[:], ALU.mult, ALU.add)
            P2c = ept(1)
            TS(P2c[:], j1[:], 1.0 / 6.0, 1e-20, ALU.mult, ALU.max)

            # z = Du / (2 * P2c^1.5)
            r_ = ept(1)
            nc.scalar.activation(r_[:], P2c[:], AFT.Sqrt)
            w_ = ept(1)
            TT(w_[:], P2c[:], r_[:], ALU.mult)
            iw = ept(1)
            nc.vector.reciprocal(iw[:], w_[:])
            zz = ept(1)
            STT(zz[:], iw[:], 0.5, Du[:], ALU.mult, ALU.mult)

            # eigenvalues via lam = t/3 + r*(sa +- ca'), sa = sin(asin(z)/3),
            # ca' = sqrt(3)*cos(asin(z)/3), both as polynomials in u = z^2
            uu = ept(1)
            TT(uu[:], zz[:], zz[:], ALU.mult)
            h1 = ept(1)
            TS(h1[:], uu[:], SA[2], SA[1], ALU.mult, ALU.add)
            h2 = ept(1)
            TT(h2[:], uu[:], h1[:], ALU.mult)
            h3 = ept(1)
            nc.vector.tensor_scalar_add(h3[:], h2[:], SA[0])
            sa = ept(1)
            TT(sa[:], zz[:], h3[:], ALU.mult)
            g1 = ept(1)
            TS(g1[:], uu[:], CA[2], CA[1], ALU.mult, ALU.add)
            g2 = ept(1)
            TT(g2[:], uu[:], g1[:], ALU.mult)
            g3 = ept(1)
            nc.vector.tensor_scalar_add(g3[:], g2[:], CA[0])
            rs = ept(1)
            TT(rs[:], r_[:], sa[:], ALU.mult)
            rc = ept(1)
            TT(rc[:], r_[:], g3[:], ALU.mult)
            m_ = ept(1)
            STT(m_[:], t1[:], 1.0 / 3.0, rs[:], ALU.mult, ALU.add)
            lam = ept(3)
            TT(lam[:, 0:1], m_[:], rc[:], ALU.add)
            TT(lam[:, 2:3], m_[:], rc[:], ALU.subtract)
            STT(lam[:, 1:2], m_[:], -2.0, t1[:], ALU.mult, ALU.add)
            lamc = ept(3)
            nc.vector.tensor_scalar_max(lamc[:], lam[:], 0.0)
            sg = ept(3)
            nc.scalar.activation(sg[:], lamc[:], AFT.Sqrt)

            # sum_s = s0 + s1 + d*s_min; rmsd = sqrt(relu(E0-2 sum_s)/n + 1e-8)
            corr = ept(1)
            TT(corr[:], sg[:, 2:3], sgn2[:], ALU.mult)
            s01 = ept(1)
            TT(s01[:], sg[:, 0:1], sg[:, 1:2], ALU.add)
            e1t = ept(1)
            STT(e1t[:], s01[:], -2.0, E0[:], ALU.mult, ALU.add)
            t11 = ept(1)
            STT(t11[:], corr[:], -2.0, e1t[:], ALU.mult, ALU.add)
            t12 = ept(1)
            nc.vector.tensor_scalar_max(t12[:], t11[:], 0.0)
            rmsd = ept(1)
            nc.scalar.activation(rmsd[:], t12[:], AFT.Sqrt, bias=eps8,
                                 scale=rn[:, 0:1])
            nc.sync.dma_start(out=o_d, in_=rmsd[:])

    nc.compile()
    return nc


def _np_dt():
    if KMODE == "fp16":
        return np.float16
    import ml_dtypes

    return ml_dtypes.float8_e4m3


def _host_z(x16, y16, n16):
    """Z [128, NBLK*112]: Z[a0, b, pl, r] = plane pl of row r atom b*128+a0."""
    m = (np.arange(MAX_ATOMS)[None, :] < n16[:, None])
    x3 = x16.reshape(ROWS, MAX_ATOMS, 3) * m[..., None]
    y3 = y16.reshape(ROWS, MAX_ATOMS, 3) * m[..., None]
    P = np.empty((PL, ROWS, MAX_ATOMS), np.float32)
    P[0:3] = np.moveaxis(x3, 2, 0)
    P[3:6] = np.moveaxis(y3, 2, 0)
    P[6] = m
    Z = P.reshape(PL, ROWS, NBLK, 128).transpose(3, 2, 0, 1)
    return np.ascontiguousarray(Z).reshape(128, NBLK * MW).astype(_np_dt())


def _host_aux(n16):
    aux = np.zeros((112, AUXW), dtype=np.float32)
    aux[:, 0:112] = np.eye(112, dtype=np.float32)
    p = np.arange(112)
    aux[p, COL_DM + (p % 16)] = 1.0
    nf = n16.astype(np.float64)
    aux[0:ROWS, COL_N] = nf
    aux[0:ROWS, COL_RN] = 1.0 / nf
    aux[0:ROWS, COL_NRN] = -1.0 / nf
    aux[0:ROWS, COL_CPM] = 2.0
    aux[0:ROWS, COL_CPM + 1] = -2.0
    aux[0:ROWS, COL_PIO] = 0.0
    aux[0:ROWS, COL_PIO + 1] = np.pi / 3.0
    aux[0:ROWS, COL_SRN] = np.sqrt(1.0 / nf)
    aux[0:ROWS, COL_EPS] = 1e-8
    return aux


def kernel(input, target, num_atoms):
    from concourse.bass_utils import run_bass_kernel_spmd

    if "nc" not in _state:
        _state["nc"] = _build()
    nc = _state["nc"]

    input = np.ascontiguousarray(np.asarray(input), dtype=np.float32)
    target = np.ascontiguousarray(np.asarray(target), dtype=np.float32)
    num_atoms = np.asarray(num_atoms)

    in_maps = []
    for c in range(NCORES):
        rs = slice(c * ROWS, (c + 1) * ROWS)
        n16 = np.asarray(num_atoms[rs])
        in_maps.append(
            {
                "z": _host_z(input[rs], target[rs], n16),
                "aux": _host_aux(n16),
            }
        )

    res = run_bass_kernel_spmd(nc, in_maps, core_ids=list(range(NCORES)))
    out = np.concatenate([r["o"].reshape(ROWS) for r in res.results])
    return out.astype(np.float32)


# revision 20
# speedup vs baseline: 3.5346x; 1.0240x over previous
"""Trainium2 Bass kernel for batched masked-Kabsch RMSD (Coords2RMSD).

Contract: kernel(**inputs) takes FULL inputs (input [128, 49152] f32,
target [128, 49152] f32, num_atoms [128] i32) and returns the FULL
output [128] f32.  Shards batch rows across 8 NeuronCores (16 rows per
core), runs one SPMD Bass program, gathers.

Device algorithm (per core), v2 "PE-Gram" design:
  - Host ships a transposed, pre-masked tensor Z[a0, b, pl, r]:
    partition a0 = atom index within a 128-atom block b, plane
    pl in {x0,x1,x2,y0,y1,y2,mask}, r = batch row.  All 17 reduction
    stats (3x3 cross-covariance, column sums, squared norms) come from
    ONE accumulated PE Gram series: for each block b,
      G += Z[:,b,:112].T @ Z[:,b,:96]        (PSUM accumulate)
    The diagonal (r==r') blocks of G are the per-row stats; cross-row
    entries are garbage that a diag-mask multiply + segmented reduce
    discards.  A set of 7 tiny selector matmuls transposes the stats to
    [16 rows, 42 channels].
  - Epilogue (per row, 16 partitions): unnormalized trigonometric
    closed-form eigenvalues of C^T C, with asin/cos evaluated as DVE
    polynomials (no arctan/sin ACT tables; only the sqrt table is used,
    preloaded during the DMA phase).
"""

import os
import sys

import numpy as np

for _p in ("/opt/trn_rl_repo", "/root/.axon_site/_ro/trn_rl_repo"):
    if os.path.isdir(_p) and _p not in sys.path:
        sys.path.insert(0, _p)

B = 128
MAX_ATOMS = 16384
N3 = 3 * MAX_ATOMS          # 49152
NCORES = 8
ROWS = B // NCORES          # 16 rows per core
NBLK = MAX_ATOMS // 128     # 128 atom blocks of 128 atoms
PL = 7                      # planes: x0 x1 x2 y0 y1 y2 mask
PLR = 6                     # rhs planes (no mask)
MW = PL * ROWS              # 112 lhsT columns
NW = PLR * ROWS             # 96 rhs columns
NT = 4                      # DMA tiles along the block dim
BPT = NBLK // NT            # 32 blocks per DMA tile

# "fp16" or "fp8" (fp8 uses DoubleRow matmuls: 2 k-tiles per pass)
KMODE = os.environ.get("K_MODE", "fp8")

AUXW = 144
COL_DM = 112      # [112, 16] diag row mask
COL_N = 128       # per-row scalars live in rows 0:16
COL_RN = 129
COL_NRN = 130
COL_SRN = 135     # sqrt(1/n)
COL_EPS = 136     # 1e-8 (rmsd bias)
COL_PC2 = 137     # (SA[2], CA[2])
COL_PC1 = 139     # (SA[1], CA[1])
COL_PC0 = 141     # (SA[0], CA[0])

# sin(asin(z)/3)  ~= z * (SA[0] + SA[1] u + SA[2] u^2), u = z^2, on [-1,1]
SA = (0.363286354, -0.129956059, 0.236283775)
# sqrt(3)*cos(asin(z)/3) ~= CA[0] + CA[1] u + CA[2] u^2
CA = (1.725367531, -0.003965617, -0.185061429)

_state = {}


def _build():
    import concourse.bacc as bacc
    import concourse.mybir as mybir
    import concourse.tile as tile

    dt = mybir.dt
    AFT = mybir.ActivationFunctionType
    ALU = mybir.AluOpType
    AX = mybir.AxisListType

    DT = dt.float16 if KMODE == "fp16" else dt.float8e4

    nc = bacc.Bacc("TRN2", target_bir_lowering=False, debug=False)

    z_d = nc.dram_tensor("z", [128, NBLK * MW], DT, kind="ExternalInput").ap()
    aux_d = nc.dram_tensor("aux", [112, AUXW], dt.float32, kind="ExternalInput").ap()
    o_d = nc.dram_tensor("o", [ROWS, 1], dt.float32, kind="ExternalOutput").ap()

    with tile.TileContext(nc) as tc:
        with (
            tc.tile_pool(name="data", bufs=1) as data_pool,
            tc.tile_pool(name="small", bufs=1) as small_pool,
            tc.tile_pool(name="ep", bufs=1) as ep_pool,
            tc.tile_pool(name="psum", bufs=1, space="PSUM") as psum_pool,
        ):
            # -------- bulk: DMA + accumulated PE Gram ------------------
            # uneven slices: small final slice so PE finishes soon after the
            # last byte lands (DMA completion sems cost +900ns each)
            SLICES = [44, 44, 32, 8] if KMODE == "fp8" else [32, 32, 32, 32]
            assert sum(SLICES) == NBLK
            zt = []
            off = 0
            for t, nb in enumerate(SLICES):
                ztile = data_pool.tile([128, nb * MW], DT, tag=f"z{t}")
                sl = slice(off * MW, (off + nb) * MW)
                nc.sync.dma_start(out=ztile[:], in_=z_d[:, sl])
                zt.append(ztile)
                off += nb

            aux = small_pool.tile([112, AUXW], dt.float32)
            nc.sync.dma_start(out=aux[:], in_=aux_d)

            # preload the sqrt activation table while DMAs stream
            warm = small_pool.tile([1, 2], dt.float32)
            nc.vector.memset(warm[:], 1.0)
            nc.scalar.activation(warm[:, 1:2], warm[:, 0:1], AFT.Sqrt)

            G = psum_pool.tile([MW, NW], dt.float32)
            if KMODE == "fp8":
                first = True
                for t, nb in enumerate(SLICES):
                    np2 = nb // 2
                    zb = zt[t][:].rearrange(
                        "p (j two c) -> p j two c", j=np2, two=2
                    )
                    for j2 in range(np2):
                        nc.tensor.matmul(
                            G[:], zb[:, j2, :, :], zb[:, j2, :, 0:NW],
                            start=first,
                            stop=(t == NT - 1 and j2 == np2 - 1),
                            perf_mode=mybir.MatmulPerfMode.DoubleRow,
                        )
                        first = False
            else:
                first = True
                for t, nb in enumerate(SLICES):
                    zb = zt[t][:].rearrange("p (j c) -> p j c", j=nb)
                    for j in range(nb):
                        nc.tensor.matmul(
                            G[:], zb[:, j, :], zb[:, j, 0:NW],
                            start=first,
                            stop=(t == NT - 1 and j == nb - 1),
                        )
                        first = False

            # -------- extract per-row stats from Gram diagonal ---------
            # R6[pl*16+r, pl'] = G[pl*16+r, pl'*16+r]
            Gm = ep_pool.tile([112, NW], dt.float32, name="Gm", tag="Gm")
            dmv = aux[:, COL_DM : COL_DM + 16]
            nc.vector.tensor_tensor(
                Gm[:].rearrange("p (c r) -> p c r", r=ROWS),
                G[:].rearrange("p (c r) -> p c r", r=ROWS),
                dmv.unsqueeze(1).broadcast_to([112, PLR, ROWS]),
                ALU.mult,
            )
            R6 = ep_pool.tile([112, PLR], dt.float32, name="R6", tag="R6")
            nc.vector.tensor_reduce(
                R6[:], Gm[:].rearrange("p (c r) -> p c r", r=ROWS), AX.X, ALU.add
            )
            # transpose stats to [16 rows, 42]: S42[r, 6*pl+pl']
            E2 = psum_pool.tile([ROWS, PL * PLR], dt.float32)
            for pl in range(PL):
                nc.tensor.matmul(
                    E2[:, PLR * pl : PLR * (pl + 1)],
                    aux[:, pl * 16 : (pl + 1) * 16],
                    R6[:],
                    start=True, stop=True,
                )
            S42 = ep_pool.tile([ROWS, PL * PLR], dt.float32, name="S42", tag="S42")
            nc.vector.tensor_scalar_mul(S42[:], E2[:], 1.0)

            # -------- epilogue ----------------------------------------
            _ep_ctr = [0]

            def ept(w):
                _ep_ctr[0] += 1
                nm = f"ep{_ep_ctr[0]}"
                return ep_pool.tile([ROWS, w], dt.float32, name=nm, tag=nm)

            TT = nc.vector.tensor_tensor
            STT = nc.vector.scalar_tensor_tensor
            TS = nc.vector.tensor_scalar

            rn = aux[0:ROWS, COL_RN : COL_RN + 1]
            nrn = aux[0:ROWS, COL_NRN : COL_NRN + 1]
            srn = aux[0:ROWS, COL_SRN : COL_SRN + 1]
            eps8 = aux[0:ROWS, COL_EPS : COL_EPS + 1]

            # channel views of S42
            s6 = S42[:, 36:42]                 # sx (3), sy (3)
            sx = S42[:, 36:39]
            sy = S42[:, 39:42]
            M3 = S42[:, 3:21].rearrange("p (k l) -> p k l", l=PLR)[:, :, 0:3]
            diag6 = S42[:].rearrange("p (a b) -> p b a", b=PL)[:, 0:1, :]

            # E0 branch on ACT (parallel with DVE mainline):
            #   ssn = (|sx|^2+|sy|^2)/n  via Square(s * sqrt(1/n)) accum
            #   sxy = Sxx + Syy          via Identity accum over diag6
            ssn = ept(1)
            scr6 = ept(PLR)
            nc.scalar.activation(scr6[:], s6, AFT.Square, scale=srn,
                                 accum_out=ssn[:])
            sxy = ept(1)
            scr6b = ept(PLR)
            nc.scalar.activation(
                scr6b[:].rearrange("p (a b) -> p a b", a=1), diag6,
                AFT.Identity, accum_out=sxy[:],
            )
            E0 = ept(1)
            TT(E0[:], sxy[:], ssn[:], ALU.subtract)

            # C = M - sx sy^T / n
            O9 = ept(9)
            o3 = O9[:].rearrange("p (k l) -> p k l", l=3)
            TT(o3, sx.unsqueeze(2).broadcast_to([ROWS, 3, 3]),
               sy.unsqueeze(1).broadcast_to([ROWS, 3, 3]), ALU.mult)
            C9 = ept(9)
            STT(C9[:].rearrange("p (k l) -> p k l", l=3), o3,
                nrn[:, 0:1], M3, ALU.mult, ALU.add)

            # det(C) partials on GPSIMD, off the DVE critical path.
            # D6/E6 = rows 1,2 of C duplicated twice (cofactors become
            # contiguous slices); computed straight from O9/M3.
            USE_POOL = os.environ.get("K_USE_POOL", "0") == "1"
            _br = nc.gpsimd if USE_POOL else nc.vector
            D6 = ept(6)
            E6 = ept(6)
            _br.scalar_tensor_tensor(
                D6[:].rearrange("p (a b) -> p a b", a=2),
                O9[:, 3:6].unsqueeze(1).broadcast_to([ROWS, 2, 3]),
                nrn[:, 0:1],
                M3[:, 1, :].unsqueeze(1).broadcast_to([ROWS, 2, 3]),
                ALU.mult, ALU.add)
            _br.scalar_tensor_tensor(
                E6[:].rearrange("p (a b) -> p a b", a=2),
                O9[:, 6:9].unsqueeze(1).broadcast_to([ROWS, 2, 3]),
                nrn[:, 0:1],
                M3[:, 2, :].unsqueeze(1).broadcast_to([ROWS, 2, 3]),
                ALU.mult, ALU.add)
            cofA = ept(3)
            cofB = ept(3)
            _br.tensor_tensor(cofA[:], D6[:, 1:4], E6[:, 2:5], ALU.mult)
            _br.tensor_tensor(cofB[:], D6[:, 2:5], E6[:, 1:4], ALU.mult)
            cof = ept(3)
            _br.tensor_tensor(cof[:], cofA[:], cofB[:], ALU.subtract)

            # A = C^T C
            W27 = ept(27)
            w3 = W27[:].rearrange("p (i j a) -> p i j a", j=3, a=3)
            cu = C9[:].rearrange("p (a i) -> p i a", i=3).unsqueeze(2)
            cv = C9[:].rearrange("p (a j) -> p j a", j=3).unsqueeze(1)
            TT(w3, cu.broadcast_to([ROWS, 3, 3, 3]),
               cv.broadcast_to([ROWS, 3, 3, 3]), ALU.mult)
            A9 = ept(9)
            nc.vector.tensor_reduce(
                A9[:].rearrange("p (i j) -> p i j", j=3), w3, AX.X, ALU.add
            )

            # t = tr(A), q = tr(A^2) = sum A9^2
            t1 = ept(1)
            nc.vector.tensor_reduce(t1[:], A9[:, 0:9:4], AX.X, ALU.add)
            f2 = ept(9)
            q1 = ept(1)
            STT(f2[:], A9[:], 1.0, A9[:], ALU.mult, ALU.mult, accum_out=q1[:])
            t2 = ept(1)
            TT(t2[:], t1[:], t1[:], ALU.mult)
            t3 = ept(1)
            TT(t3[:], t2[:], t1[:], ALU.mult)

            det3 = ept(3)
            detC = ept(1)
            STT(det3[:], C9[:, 0:3], 1.0, cof[:], ALU.mult, ALU.mult,
                accum_out=detC[:])
            dA = ept(1)
            TT(dA[:], detC[:], detC[:], ALU.mult)
            # sign(detC) -> {-1, +1} (for the reflection term d*s_min)
            sgn = ept(1)
            TS(sgn[:], detC[:], 0.0, None, ALU.is_ge)
            sgn2 = ept(1)
            TS(sgn2[:], sgn[:], 2.0, -1.0, ALU.mult, ALU.add)

            # Du = det(A - (t/3) I) = detC^2 + t*q/6 - (5/54) t^3
            tq = ept(1)
            TT(tq[:], t1[:], q1[:], ALU.mult)
            Du1 = ept(1)
            STT(Du1[:], tq[:], 1.0 / 6.0, dA[:], ALU.mult, ALU.add)
            Du = ept(1)
            STT(Du[:], t3[:], -5.0 / 54.0, Du1[:], ALU.mult, ALU.add)
            # P2c = max((q - t^2/3)/6, eps)
            j1 = ept(1)
            STT(j1[:], t2[:], -1.0 / 3.0, q1[:], ALU.mult, ALU.add)
            P2c = ept(1)
            TS(P2c[:], j1[:], 1.0 / 6.0, 1e-20, ALU.mult, ALU.max)

            # z = Du / (2 * P2c^1.5)
            r_ = ept(1)
            nc.scalar.activation(r_[:], P2c[:], AFT.Sqrt)
            w_ = ept(1)
            TT(w_[:], P2c[:], r_[:], ALU.mult)
            iw = ept(1)
            nc.vector.reciprocal(iw[:], w_[:])
            zz = ept(1)
            STT(zz[:], iw[:], 0.5, Du[:], ALU.mult, ALU.mult)

            # eigenvalues via lam = t/3 + r*(sa +- ca'), sa = sin(asin(z)/3),
            # ca' = sqrt(3)*cos(asin(z)/3); both deg-2 polys in u = z^2,
            # evaluated together on a [16,2] tile with per-column coeffs
            zb2 = zz[:].broadcast_to([ROWS, 2])
            uu2 = ept(2)
            TT(uu2[:], zb2, zb2, ALU.mult)
            pm1 = ept(2)
            TT(pm1[:], uu2[:], aux[0:ROWS, COL_PC2 : COL_PC2 + 2], ALU.mult)
            pa1 = ept(2)
            TT(pa1[:], pm1[:], aux[0:ROWS, COL_PC1 : COL_PC1 + 2], ALU.add)
            pm2 = ept(2)
            TT(pm2[:], pa1[:], uu2[:], ALU.mult)
            pa2 = ept(2)
            TT(pa2[:], pm2[:], aux[0:ROWS, COL_PC0 : COL_PC0 + 2], ALU.add)
            zr = ept(1)
            TT(zr[:], zz[:], r_[:], ALU.mult)
            rs = ept(1)
            TT(rs[:], zr[:], pa2[:, 0:1], ALU.mult)
            rc = ept(1)
            TT(rc[:], r_[:], pa2[:, 1:2], ALU.mult)
            m_ = ept(1)
            STT(m_[:], t1[:], 1.0 / 3.0, rs[:], ALU.mult, ALU.add)
            lam = ept(3)
            TT(lam[:, 0:1], m_[:], rc[:], ALU.add)
            TT(lam[:, 2:3], m_[:], rc[:], ALU.subtract)
            STT(lam[:, 1:2], m_[:], -2.0, t1[:], ALU.mult, ALU.add)
            lamc = ept(3)
            nc.vector.tensor_scalar_max(lamc[:], lam[:], 0.0)
            sg = ept(3)
            nc.scalar.activation(sg[:], lamc[:], AFT.Sqrt)

            # sum_s = s0 + s1 + d*s_min; rmsd = sqrt(relu(E0-2 sum_s)/n + 1e-8)
            corr = ept(1)
            TT(corr[:], sg[:, 2:3], sgn2[:], ALU.mult)
            s01 = ept(1)
            TT(s01[:], sg[:, 0:1], sg[:, 1:2], ALU.add)
            e1t = ept(1)
            STT(e1t[:], s01[:], -2.0, E0[:], ALU.mult, ALU.add)
            t11 = ept(1)
            STT(t11[:], corr[:], -2.0, e1t[:], ALU.mult, ALU.add)
            t12 = ept(1)
            nc.vector.tensor_scalar_max(t12[:], t11[:], 0.0)
            rmsd = ept(1)
            nc.scalar.activation(rmsd[:], t12[:], AFT.Sqrt, bias=eps8,
                                 scale=rn[:, 0:1])
            nc.sync.dma_start(out=o_d, in_=rmsd[:])

    nc.compile()
    return nc


def _np_dt():
    if KMODE == "fp16":
        return np.float16
    import ml_dtypes

    return ml_dtypes.float8_e4m3


def _host_z(x16, y16, n16):
    """Z [128, NBLK*112]: Z[a0, b, pl, r] = plane pl of row r atom b*128+a0."""
    m = (np.arange(MAX_ATOMS)[None, :] < n16[:, None])
    x3 = x16.reshape(ROWS, MAX_ATOMS, 3) * m[..., None]
    y3 = y16.reshape(ROWS, MAX_ATOMS, 3) * m[..., None]
    P = np.empty((PL, ROWS, MAX_ATOMS), np.float32)
    P[0:3] = np.moveaxis(x3, 2, 0)
    P[3:6] = np.moveaxis(y3, 2, 0)
    P[6] = m
    Z = P.reshape(PL, ROWS, NBLK, 128).transpose(3, 2, 0, 1)
    return np.ascontiguousarray(Z).reshape(128, NBLK * MW).astype(_np_dt())


def _host_aux(n16):
    aux = np.zeros((112, AUXW), dtype=np.float32)
    aux[:, 0:112] = np.eye(112, dtype=np.float32)
    p = np.arange(112)
    aux[p, COL_DM + (p % 16)] = 1.0
    nf = n16.astype(np.float64)
    aux[0:ROWS, COL_N] = nf
    aux[0:ROWS, COL_RN] = 1.0 / nf
    aux[0:ROWS, COL_NRN] = -1.0 / nf
    aux[0:ROWS, COL_SRN] = np.sqrt(1.0 / nf)
    aux[0:ROWS, COL_EPS] = 1e-8
    aux[0:ROWS, COL_PC2] = SA[2]
    aux[0:ROWS, COL_PC2 + 1] = CA[2]
    aux[0:ROWS, COL_PC1] = SA[1]
    aux[0:ROWS, COL_PC1 + 1] = CA[1]
    aux[0:ROWS, COL_PC0] = SA[0]
    aux[0:ROWS, COL_PC0 + 1] = CA[0]
    return aux


def kernel(input, target, num_atoms):
    from concourse.bass_utils import run_bass_kernel_spmd

    if "nc" not in _state:
        _state["nc"] = _build()
    nc = _state["nc"]

    input = np.ascontiguousarray(np.asarray(input), dtype=np.float32)
    target = np.ascontiguousarray(np.asarray(target), dtype=np.float32)
    num_atoms = np.asarray(num_atoms)

    in_maps = []
    for c in range(NCORES):
        rs = slice(c * ROWS, (c + 1) * ROWS)
        n16 = np.asarray(num_atoms[rs])
        in_maps.append(
            {
                "z": _host_z(input[rs], target[rs], n16),
                "aux": _host_aux(n16),
            }
        )

    res = run_bass_kernel_spmd(nc, in_maps, core_ids=list(range(NCORES)))
    out = np.concatenate([r["o"].reshape(ROWS) for r in res.results])
    return out.astype(np.float32)


# revision 25
# speedup vs baseline: 3.6560x; 1.0343x over previous
"""Trainium2 Bass kernel for batched masked-Kabsch RMSD (Coords2RMSD).

Contract: kernel(**inputs) takes FULL inputs (input [128, 49152] f32,
target [128, 49152] f32, num_atoms [128] i32) and returns the FULL
output [128] f32.  Shards batch rows across 8 NeuronCores (16 rows per
core), runs one SPMD Bass program, gathers.

Device algorithm (per core), v2 "PE-Gram" design:
  - Host ships a transposed, pre-masked tensor Z[a0, b, pl, r]:
    partition a0 = atom index within a 128-atom block b, plane
    pl in {x0,x1,x2,y0,y1,y2,mask}, r = batch row.  All 17 reduction
    stats (3x3 cross-covariance, column sums, squared norms) come from
    ONE accumulated PE Gram series: for each block b,
      G += Z[:,b,:112].T @ Z[:,b,:96]        (PSUM accumulate)
    The diagonal (r==r') blocks of G are the per-row stats; cross-row
    entries are garbage that a diag-mask multiply + segmented reduce
    discards.  A set of 7 tiny selector matmuls transposes the stats to
    [16 rows, 42 channels].
  - Epilogue (per row, 16 partitions): unnormalized trigonometric
    closed-form eigenvalues of C^T C, with asin/cos evaluated as DVE
    polynomials (no arctan/sin ACT tables; only the sqrt table is used,
    preloaded during the DMA phase).
"""

import os
import sys

import numpy as np

for _p in ("/opt/trn_rl_repo", "/root/.axon_site/_ro/trn_rl_repo"):
    if os.path.isdir(_p) and _p not in sys.path:
        sys.path.insert(0, _p)

B = 128
MAX_ATOMS = 16384
N3 = 3 * MAX_ATOMS          # 49152
NCORES = 8
ROWS = B // NCORES          # 16 rows per core
NBLK = MAX_ATOMS // 128     # 128 atom blocks of 128 atoms
PL = 7                      # planes: x0 x1 x2 y0 y1 y2 mask
PLR = 6                     # rhs planes (no mask)
MW = PL * ROWS              # 112 lhsT columns
NW = PLR * ROWS             # 96 rhs columns
NT = 4                      # DMA tiles along the block dim
BPT = NBLK // NT            # 32 blocks per DMA tile

# "fp16" or "fp8" (fp8 uses DoubleRow matmuls: 2 k-tiles per pass)
KMODE = os.environ.get("K_MODE", "fp8")

AUXW = 152
COL_DM = 112      # [112, 16] diag row mask
COL_N = 128       # per-row scalars live in rows 0:16
COL_RN = 129
COL_NRN = 130
COL_SRN = 135     # sqrt(1/n)
COL_EPS = 136     # 1e-8 (rmsd bias)
COL_PC2 = 137     # (SA[2], CA[2])
COL_PC1 = 139     # (SA[1], CA[1])
COL_PC0 = 141     # (SA[0], CA[0])
COL_C6 = 143      # 1/6  (ACT scale)
COL_C54 = 144     # -5/54 (ACT scale)

# sin(asin(z)/3)  ~= z * (SA[0] + SA[1] u + SA[2] u^2), u = z^2, on [-1,1]
SA = (0.363286354, -0.129956059, 0.236283775)
# sqrt(3)*cos(asin(z)/3) ~= CA[0] + CA[1] u + CA[2] u^2
CA = (1.725367531, -0.003965617, -0.185061429)

_state = {}


def _build():
    import concourse.bacc as bacc
    import concourse.mybir as mybir
    import concourse.tile as tile

    dt = mybir.dt
    AFT = mybir.ActivationFunctionType
    ALU = mybir.AluOpType
    AX = mybir.AxisListType

    DT = dt.float16 if KMODE == "fp16" else dt.float8e4

    nc = bacc.Bacc("TRN2", target_bir_lowering=False, debug=False)

    z_d = nc.dram_tensor("z", [128, NBLK * MW], DT, kind="ExternalInput").ap()
    aux_d = nc.dram_tensor("aux", [112, AUXW], dt.float32, kind="ExternalInput").ap()
    o_d = nc.dram_tensor("o", [ROWS, 1], dt.float32, kind="ExternalOutput").ap()

    with tile.TileContext(nc) as tc:
        with (
            tc.tile_pool(name="data", bufs=1) as data_pool,
            tc.tile_pool(name="small", bufs=1) as small_pool,
            tc.tile_pool(name="ep", bufs=1) as ep_pool,
            tc.tile_pool(name="psum", bufs=1, space="PSUM") as psum_pool,
        ):
            # -------- bulk: DMA + accumulated PE Gram ------------------
            # uneven slices: small final slice so PE finishes soon after the
            # last byte lands (DMA completion sems cost +900ns each)
            SLICES = [44, 44, 32, 8] if KMODE == "fp8" else [32, 32, 32, 32]
            assert sum(SLICES) == NBLK
            zt = []
            off = 0
            for t, nb in enumerate(SLICES):
                ztile = data_pool.tile([128, nb * MW], DT, tag=f"z{t}")
                sl = slice(off * MW, (off + nb) * MW)
                nc.sync.dma_start(out=ztile[:], in_=z_d[:, sl])
                zt.append(ztile)
                off += nb

            aux = small_pool.tile([112, AUXW], dt.float32)
            nc.sync.dma_start(out=aux[:], in_=aux_d)

            # preload the sqrt activation table while DMAs stream
            warm = small_pool.tile([1, 2], dt.float32)
            nc.vector.memset(warm[:], 1.0)
            nc.scalar.activation(warm[:, 1:2], warm[:, 0:1], AFT.Sqrt)

            G = psum_pool.tile([MW, NW], dt.float32)
            if KMODE == "fp8":
                first = True
                for t, nb in enumerate(SLICES):
                    np2 = nb // 2
                    zb = zt[t][:].rearrange(
                        "p (j two c) -> p j two c", j=np2, two=2
                    )
                    for j2 in range(np2):
                        nc.tensor.matmul(
                            G[:], zb[:, j2, :, :], zb[:, j2, :, 0:NW],
                            start=first,
                            stop=(t == NT - 1 and j2 == np2 - 1),
                            perf_mode=mybir.MatmulPerfMode.DoubleRow,
                        )
                        first = False
            else:
                first = True
                for t, nb in enumerate(SLICES):
                    zb = zt[t][:].rearrange("p (j c) -> p j c", j=nb)
                    for j in range(nb):
                        nc.tensor.matmul(
                            G[:], zb[:, j, :], zb[:, j, 0:NW],
                            start=first,
                            stop=(t == NT - 1 and j == nb - 1),
                        )
                        first = False

            # -------- extract per-row stats from Gram diagonal ---------
            # R6[pl*16+r, pl'] = G[pl*16+r, pl'*16+r]
            Gm = ep_pool.tile([112, NW], dt.float32, name="Gm", tag="Gm")
            dmv = aux[:, COL_DM : COL_DM + 16]
            nc.vector.tensor_tensor(
                Gm[:].rearrange("p (c r) -> p c r", r=ROWS),
                G[:].rearrange("p (c r) -> p c r", r=ROWS),
                dmv.unsqueeze(1).broadcast_to([112, PLR, ROWS]),
                ALU.mult,
            )
            R6 = ep_pool.tile([112, PLR], dt.float32, name="R6", tag="R6")
            nc.vector.tensor_reduce(
                R6[:], Gm[:].rearrange("p (c r) -> p c r", r=ROWS), AX.X, ALU.add
            )
            # transpose stats to [16 rows, 42]: S42[r, 6*pl+pl']
            E2 = psum_pool.tile([ROWS, PL * PLR], dt.float32)
            for pl in range(PL):
                nc.tensor.matmul(
                    E2[:, PLR * pl : PLR * (pl + 1)],
                    aux[:, pl * 16 : (pl + 1) * 16],
                    R6[:],
                    start=True, stop=True,
                )
            S42 = ep_pool.tile([ROWS, PL * PLR], dt.float32, name="S42", tag="S42")
            nc.vector.tensor_scalar_mul(S42[:], E2[:], 1.0)

            # -------- epilogue ----------------------------------------
            _ep_ctr = [0]

            def ept(w):
                _ep_ctr[0] += 1
                nm = f"ep{_ep_ctr[0]}"
                return ep_pool.tile([ROWS, w], dt.float32, name=nm, tag=nm)

            TT = nc.vector.tensor_tensor
            STT = nc.vector.scalar_tensor_tensor
            TS = nc.vector.tensor_scalar

            rn = aux[0:ROWS, COL_RN : COL_RN + 1]
            nrn = aux[0:ROWS, COL_NRN : COL_NRN + 1]
            srn = aux[0:ROWS, COL_SRN : COL_SRN + 1]
            eps8 = aux[0:ROWS, COL_EPS : COL_EPS + 1]

            # channel views of S42
            s6 = S42[:, 36:42]                 # sx (3), sy (3)
            sx = S42[:, 36:39]
            sy = S42[:, 39:42]
            M3 = S42[:, 3:21].rearrange("p (k l) -> p k l", l=PLR)[:, :, 0:3]
            diag6 = S42[:].rearrange("p (a b) -> p b a", b=PL)[:, 0:1, :]

            # E0 branch on ACT (parallel with DVE mainline):
            #   ssn = (|sx|^2+|sy|^2)/n  via Square(s * sqrt(1/n)) accum
            #   sxy = Sxx + Syy          via Identity accum over diag6
            ssn = ept(1)
            scr6 = ept(PLR)
            nc.scalar.activation(scr6[:], s6, AFT.Square, scale=srn,
                                 accum_out=ssn[:])
            sxy = ept(1)
            scr6b = ept(PLR)
            nc.scalar.activation(
                scr6b[:].rearrange("p (a b) -> p a b", a=1), diag6,
                AFT.Identity, accum_out=sxy[:],
            )
            E0 = ept(1)
            TT(E0[:], sxy[:], ssn[:], ALU.subtract)

            # C = M - sx sy^T / n
            O9 = ept(9)
            o3 = O9[:].rearrange("p (k l) -> p k l", l=3)
            TT(o3, sx.unsqueeze(2).broadcast_to([ROWS, 3, 3]),
               sy.unsqueeze(1).broadcast_to([ROWS, 3, 3]), ALU.mult)
            C9 = ept(9)
            STT(C9[:].rearrange("p (k l) -> p k l", l=3), o3,
                nrn[:, 0:1], M3, ALU.mult, ALU.add)

            # det(C) partials on GPSIMD, off the DVE critical path.
            # D6/E6 = rows 1,2 of C duplicated twice (cofactors become
            # contiguous slices); computed straight from O9/M3.
            USE_POOL = os.environ.get("K_USE_POOL", "0") == "1"
            _br = nc.gpsimd if USE_POOL else nc.vector
            D6 = ept(6)
            E6 = ept(6)
            _br.scalar_tensor_tensor(
                D6[:].rearrange("p (a b) -> p a b", a=2),
                O9[:, 3:6].unsqueeze(1).broadcast_to([ROWS, 2, 3]),
                nrn[:, 0:1],
                M3[:, 1, :].unsqueeze(1).broadcast_to([ROWS, 2, 3]),
                ALU.mult, ALU.add)
            _br.scalar_tensor_tensor(
                E6[:].rearrange("p (a b) -> p a b", a=2),
                O9[:, 6:9].unsqueeze(1).broadcast_to([ROWS, 2, 3]),
                nrn[:, 0:1],
                M3[:, 2, :].unsqueeze(1).broadcast_to([ROWS, 2, 3]),
                ALU.mult, ALU.add)
            cofA = ept(3)
            cofB = ept(3)
            _br.tensor_tensor(cofA[:], D6[:, 1:4], E6[:, 2:5], ALU.mult)
            _br.tensor_tensor(cofB[:], D6[:, 2:5], E6[:, 1:4], ALU.mult)
            cof = ept(3)
            _br.tensor_tensor(cof[:], cofA[:], cofB[:], ALU.subtract)

            # A = C^T C
            W27 = ept(27)
            w3 = W27[:].rearrange("p (i j a) -> p i j a", j=3, a=3)
            cu = C9[:].rearrange("p (a i) -> p i a", i=3).unsqueeze(2)
            cv = C9[:].rearrange("p (a j) -> p j a", j=3).unsqueeze(1)
            TT(w3, cu.broadcast_to([ROWS, 3, 3, 3]),
               cv.broadcast_to([ROWS, 3, 3, 3]), ALU.mult)
            A9 = ept(9)
            nc.vector.tensor_reduce(
                A9[:].rearrange("p (i j) -> p i j", j=3), w3, AX.X, ALU.add
            )

            # t = tr(A), q = tr(A^2) = sum A9^2
            t1 = ept(1)
            nc.vector.tensor_reduce(t1[:], A9[:, 0:9:4], AX.X, ALU.add)
            f2 = ept(9)
            q1 = ept(1)
            STT(f2[:], A9[:], 1.0, A9[:], ALU.mult, ALU.mult, accum_out=q1[:])
            t2 = ept(1)
            TT(t2[:], t1[:], t1[:], ALU.mult)

            det3 = ept(3)
            detC = ept(1)
            STT(det3[:], C9[:, 0:3], 1.0, cof[:], ALU.mult, ALU.mult,
                accum_out=detC[:])

            # Du = det(A - (t/3) I) = detC^2 + t*q/6 - (5/54) t^3
            # whole branch on ACT (Square / Identity with AP scale+bias),
            # parallel with the DVE mainline
            c6 = aux[0:ROWS, COL_C6 : COL_C6 + 1]
            c54 = aux[0:ROWS, COL_C54 : COL_C54 + 1]
            t3 = ept(1)
            nc.scalar.activation(t3[:], t2[:], AFT.Copy, scale=t1[:, 0:1])
            dA = ept(1)
            nc.scalar.activation(dA[:], detC[:], AFT.Square)
            tq = ept(1)
            nc.scalar.activation(tq[:], q1[:], AFT.Copy, scale=t1[:, 0:1])
            Du1 = ept(1)
            nc.scalar.activation(Du1[:], tq[:], AFT.Identity, scale=c6,
                                 bias=dA[:, 0:1])
            Du = ept(1)
            nc.scalar.activation(Du[:], t3[:], AFT.Identity, scale=c54,
                                 bias=Du1[:, 0:1])

            # P2c = max((q - t^2/3)/6, eps)
            j1 = ept(1)
            STT(j1[:], t2[:], -1.0 / 3.0, q1[:], ALU.mult, ALU.add)
            P2c = ept(1)
            TS(P2c[:], j1[:], 1.0 / 6.0, 1e-20, ALU.mult, ALU.max)

            # z = Du / (2 * P2c^1.5)
            r_ = ept(1)
            nc.scalar.activation(r_[:], P2c[:], AFT.Sqrt)
            w_ = ept(1)
            nc.scalar.activation(w_[:], P2c[:], AFT.Copy, scale=r_[:, 0:1])
            iw = ept(1)
            nc.vector.reciprocal(iw[:], w_[:])
            zz = ept(1)
            STT(zz[:], iw[:], 0.5, Du[:], ALU.mult, ALU.mult)

            # eigenvalues via lam = t/3 + r*(sa +- ca'), sa = sin(asin(z)/3),
            # ca' = sqrt(3)*cos(asin(z)/3); both deg-2 polys in u = z^2,
            # evaluated together on a [16,2] tile with per-column coeffs
            zb2 = zz[:].broadcast_to([ROWS, 2])
            uu2 = ept(2)
            TT(uu2[:], zb2, zb2, ALU.mult)
            pm1 = ept(2)
            TT(pm1[:], uu2[:], aux[0:ROWS, COL_PC2 : COL_PC2 + 2], ALU.mult)
            pa1 = ept(2)
            TT(pa1[:], pm1[:], aux[0:ROWS, COL_PC1 : COL_PC1 + 2], ALU.add)
            pm2 = ept(2)
            TT(pm2[:], pa1[:], uu2[:], ALU.mult)
            pa2 = ept(2)
            TT(pa2[:], pm2[:], aux[0:ROWS, COL_PC0 : COL_PC0 + 2], ALU.add)
            zr = ept(1)
            TT(zr[:], zz[:], r_[:], ALU.mult)
            rs = ept(1)
            TT(rs[:], zr[:], pa2[:, 0:1], ALU.mult)
            rc = ept(1)
            TT(rc[:], r_[:], pa2[:, 1:2], ALU.mult)
            m_ = ept(1)
            STT(m_[:], t1[:], 1.0 / 3.0, rs[:], ALU.mult, ALU.add)
            lam = ept(3)
            TT(lam[:, 0:1], m_[:], rc[:], ALU.add)
            TT(lam[:, 2:3], m_[:], rc[:], ALU.subtract)
            STT(lam[:, 1:2], m_[:], -2.0, t1[:], ALU.mult, ALU.add)
            lamc = ept(3)
            nc.vector.tensor_scalar_max(lamc[:], lam[:], 0.0)
            sg = ept(3)
            nc.scalar.activation(sg[:], lamc[:], AFT.Sqrt)

            # sum_s = s0 + s1 + d*s_min; rmsd = sqrt(relu(E0-2 sum_s)/n + 1e-8)
            # d*s_min via copysign: OR the sign bit of detC into s_min
            u32 = dt.uint32
            sb = ept(1)
            TS(sb[:].bitcast(u32), detC[:].bitcast(u32), 0x80000000, None,
               ALU.bitwise_and)
            corr = ept(1)
            TT(corr[:].bitcast(u32), sg[:, 2:3].bitcast(u32), sb[:].bitcast(u32),
               ALU.bitwise_or)
            s01 = ept(1)
            TT(s01[:], sg[:, 0:1], sg[:, 1:2], ALU.add)
            e1t = ept(1)
            STT(e1t[:], s01[:], -2.0, E0[:], ALU.mult, ALU.add)
            t11 = ept(1)
            STT(t11[:], corr[:], -2.0, e1t[:], ALU.mult, ALU.add)
            t12 = ept(1)
            nc.vector.tensor_scalar_max(t12[:], t11[:], 0.0)
            rmsd = ept(1)
            nc.scalar.activation(rmsd[:], t12[:], AFT.Sqrt, bias=eps8,
                                 scale=rn[:, 0:1])
            nc.sync.dma_start(out=o_d, in_=rmsd[:])

    nc.compile()
    return nc


def _np_dt():
    if KMODE == "fp16":
        return np.float16
    import ml_dtypes

    return ml_dtypes.float8_e4m3


def _host_z(x16, y16, n16):
    """Z [128, NBLK*112]: Z[a0, b, pl, r] = plane pl of row r atom b*128+a0."""
    m = (np.arange(MAX_ATOMS)[None, :] < n16[:, None])
    x3 = x16.reshape(ROWS, MAX_ATOMS, 3) * m[..., None]
    y3 = y16.reshape(ROWS, MAX_ATOMS, 3) * m[..., None]
    P = np.empty((PL, ROWS, MAX_ATOMS), np.float32)
    P[0:3] = np.moveaxis(x3, 2, 0)
    P[3:6] = np.moveaxis(y3, 2, 0)
    P[6] = m
    Z = P.reshape(PL, ROWS, NBLK, 128).transpose(3, 2, 0, 1)
    return np.ascontiguousarray(Z).reshape(128, NBLK * MW).astype(_np_dt())


def _host_aux(n16):
    aux = np.zeros((112, AUXW), dtype=np.float32)
    aux[:, 0:112] = np.eye(112, dtype=np.float32)
    p = np.arange(112)
    aux[p, COL_DM + (p % 16)] = 1.0
    nf = n16.astype(np.float64)
    aux[0:ROWS, COL_N] = nf
    aux[0:ROWS, COL_RN] = 1.0 / nf
    aux[0:ROWS, COL_NRN] = -1.0 / nf
    aux[0:ROWS, COL_SRN] = np.sqrt(1.0 / nf)
    aux[0:ROWS, COL_EPS] = 1e-8
    aux[0:ROWS, COL_PC2] = SA[2]
    aux[0:ROWS, COL_PC2 + 1] = CA[2]
    aux[0:ROWS, COL_PC1] = SA[1]
    aux[0:ROWS, COL_PC1 + 1] = CA[1]
    aux[0:ROWS, COL_PC0] = SA[0]
    aux[0:ROWS, COL_PC0 + 1] = CA[0]
    aux[0:ROWS, COL_C6] = 1.0 / 6.0
    aux[0:ROWS, COL_C54] = -5.0 / 54.0
    return aux


def kernel(input, target, num_atoms):
    from concourse.bass_utils import run_bass_kernel_spmd

    if "nc" not in _state:
        _state["nc"] = _build()
    nc = _state["nc"]

    input = np.ascontiguousarray(np.asarray(input), dtype=np.float32)
    target = np.ascontiguousarray(np.asarray(target), dtype=np.float32)
    num_atoms = np.asarray(num_atoms)

    in_maps = []
    for c in range(NCORES):
        rs = slice(c * ROWS, (c + 1) * ROWS)
        n16 = np.asarray(num_atoms[rs])
        in_maps.append(
            {
                "z": _host_z(input[rs], target[rs], n16),
                "aux": _host_aux(n16),
            }
        )

    res = run_bass_kernel_spmd(nc, in_maps, core_ids=list(range(NCORES)))
    out = np.concatenate([r["o"].reshape(ROWS) for r in res.results])
    return out.astype(np.float32)
